# revision 30
# baseline (speedup 1.0000x reference)
"""Trainium2 Bass kernel for a GPT-2-style transformer block (v3, fp8).

B=4, T=1024, C=768, H=12 heads (HD=64). 8 NeuronCores.
2 cores per sequence; each core owns 512 query tokens (block-permuted).

v3 vs v2 (bf16 baseline):
- All big matmuls fp8e4 DoubleRow (contraction in 256-pairs): ~1.84x PE.
- Weights host-scaled into fp8 range (x64/x128/x16); scales folded into
  the exp scale, sigmoid scale, and epilogue scalar ops. No unscale ops.
- LN applied as xs = x*rstd only; the -mu*rstd and bias terms enter each
  projection as a K=2 appended matmul (wsum (x) nmr + bias (x) ones).
- Causal masks folded into the score PSUM via appended matmuls
  (tril-bias x identity; per-core flag row x ones) -> no DVE mask work.
- Softmax denominators via fp8 ones-matmuls into a shared [33,TQ] PSUM;
  reciprocal row broadcast to 128 partitions via SBUF->SBUF DMA.
- gelu: xb on DVE, squares on gpsimd, sigmoid on ACT, h1 on DVE.
- proj matmuls interleaved into the FC loop (6-bank persistent PSUM).
"""

import numpy as np
import ml_dtypes

P = 128
B, T, C, H = 4, 1024, 768, 12
HD = C // H        # 64
CJ = C // P        # 6 chunks
NT = T // P        # 8 token blocks
TQ = 512           # own query tokens per core
NQT = TQ // P      # 4 q blocks
FC = 4 * C         # 3072
FCJ = FC // P      # 24
NPAIR = H // 2     # 6 head pairs
GA2 = 0.035677408136300527   # sqrt(2/pi)*0.044715
N_CORES = 8

SQ = 64.0     # q,k weight scale
SV = 16.0     # v weight scale
SO = 128.0    # Wo scale
SF = 64.0     # Wfc scale
SP = 128.0    # Wproj scale
MASKNEG = -1e9

_CACHED = {}


def _build_nc():
    import concourse.bass as bass
    from concourse import bacc, mybir
    import concourse.tile as tile
    from contextlib import ExitStack

    F32 = mybir.dt.float32
    BF16 = mybir.dt.bfloat16
    FP8 = mybir.dt.float8e4
    AF = mybir.ActivationFunctionType
    ALU = mybir.AluOpType
    DR = mybir.MatmulPerfMode.DoubleRow

    nc = bacc.Bacc()

    xt_d = nc.declare_dram_parameter("xt", [C, T], BF16, isOutput=False)
    wv_d = nc.declare_dram_parameter("wv", [P, CJ, C], FP8, isOutput=False)
    wk_d = nc.declare_dram_parameter("wk", [P, CJ, C], FP8, isOutput=False)
    wq_d = nc.declare_dram_parameter("wq", [P, CJ, C], FP8, isOutput=False)
    wo_d = nc.declare_dram_parameter("wo", [P, CJ, C], FP8, isOutput=False)
    wfc_d = nc.declare_dram_parameter("wfc", [P, CJ, FC], FP8, isOutput=False)
    wpj_d = nc.declare_dram_parameter("wpj", [P, FCJ, C], FP8, isOutput=False)
    kapp_d = nc.declare_dram_parameter("kapp", [2, C], BF16, isOutput=False)
    qapp_d = nc.declare_dram_parameter("qapp", [2, C], BF16, isOutput=False)
    vapp_d = nc.declare_dram_parameter("vapp", [2, C], BF16, isOutput=False)
    woapp_d = nc.declare_dram_parameter("woapp", [1, C], BF16, isOutput=False)
    pjapp_d = nc.declare_dram_parameter("pjapp", [1, C], BF16, isOutput=False)
    bfcb_d = nc.declare_dram_parameter("bfcb", [P, FCJ], F32, isOutput=False)
    msk_d = nc.declare_dram_parameter("msk", [P, 2 * P + 384 + 256], FP8, isOutput=False)
    out_d = nc.declare_dram_parameter("out", [C, TQ], F32, isOutput=True)
    import os
    dbg = os.environ.get("KV3_DEBUG") == "1"
    if dbg:
        dbg_xh = nc.declare_dram_parameter("dbg_xh", [P, CJ * T], FP8, isOutput=True)
        dbg_k = nc.declare_dram_parameter("dbg_k", [NPAIR * P, T], BF16, isOutput=True)
        dbg_q = nc.declare_dram_parameter("dbg_q", [NPAIR * P, TQ], BF16, isOutput=True)
        dbg_vdx = nc.declare_dram_parameter("dbg_vdx", [P, NT * H * P], FP8, isOutput=True)
        dbg_yT = nc.declare_dram_parameter("dbg_yT", [P, CJ * TQ], FP8, isOutput=True)
        dbg_x1 = nc.declare_dram_parameter("dbg_x1", [CJ * P, TQ], BF16, isOutput=True)
        dbg_xh2 = nc.declare_dram_parameter("dbg_xh2", [P, CJ * TQ], FP8, isOutput=True)
        dbg_h1 = nc.declare_dram_parameter("dbg_h1", [P, FCJ * TQ], FP8, isOutput=True)
        dbg_avd = nc.declare_dram_parameter("dbg_avd", [P, NPAIR * 2 * TQ], F32, isOutput=True)
        dbg_rd = nc.declare_dram_parameter("dbg_rd", [HD, NPAIR * 2 * TQ], F32, isOutput=True)

    with tile.TileContext(nc) as tc, ExitStack() as ctx:
        persist = ctx.enter_context(tc.tile_pool(name="persist", bufs=1))

        # ---------- DMAs spread across engine queues ----------
        xbf = [persist.tile([P, T], BF16, tag=f"xbf{m}", name=f"xbf{m}")
               for m in range(CJ)]
        for m in range(CJ):
            eng = nc.sync if m % 2 == 0 else nc.scalar
            eng.dma_start(xbf[m], xt_d[m * P:(m + 1) * P, :])
        wv_t = persist.tile([P, CJ, C], FP8, tag="wv")
        nc.sync.dma_start(wv_t, wv_d[:, :, :])
        wk_t = persist.tile([P, CJ, C], FP8, tag="wk")
        nc.sync.dma_start(wk_t, wk_d[:, :, :])
        wq_t = persist.tile([P, CJ, C], FP8, tag="wq")
        nc.scalar.dma_start(wq_t, wq_d[:, :, :])
        wo_t = persist.tile([P, CJ, C], FP8, tag="wo")
        nc.scalar.dma_start(wo_t, wo_d[:, :, :])
        wfc_t = persist.tile([P, CJ, FC], FP8, tag="wfc")
        nc.scalar.dma_start(wfc_t, wfc_d[:, :, :])
        wpj_t = persist.tile([P, FCJ, C], FP8, tag="wpj")
        nc.gpsimd.dma_start(wpj_t, wpj_d[:, :, :])
        kapp_t = persist.tile([2, C], BF16, tag="kapp")
        nc.scalar.dma_start(kapp_t, kapp_d[:, :])
        qapp_t = persist.tile([2, C], BF16, tag="qapp")
        nc.scalar.dma_start(qapp_t, qapp_d[:, :])
        vapp_t = persist.tile([2, C], BF16, tag="vapp")
        nc.scalar.dma_start(vapp_t, vapp_d[:, :])
        woapp_t = persist.tile([1, C], BF16, tag="woapp")
        nc.scalar.dma_start(woapp_t, woapp_d[:, :])
        pjapp_t = persist.tile([1, C], BF16, tag="pjapp")
        nc.scalar.dma_start(pjapp_t, pjapp_d[:, :])
        bfcb_t = persist.tile([P, FCJ], F32, tag="bfcb")
        nc.scalar.dma_start(bfcb_t, bfcb_d[:, :])
        msk_t = persist.tile([P, 2 * P + 384 + 256], FP8, tag="msk")
        nc.scalar.dma_start(msk_t, msk_d[:, :])

        # ---------- constants ----------
        onesc_bf = persist.tile([P, 1], BF16, tag="onescbf")
        nc.vector.memset(onesc_bf, 1.0)
        ones_row = persist.tile([1, P], BF16, tag="onesrow")
        nc.vector.memset(ones_row, 1.0)
        mones_row = persist.tile([1, P], BF16, tag="monesrow")
        nc.vector.memset(mones_row, -1.0)
        ones_hd = persist.tile([1, HD], BF16, tag="oneshd")
        nc.vector.memset(ones_hd, 1.0)
        eps1 = persist.tile([1, 1], F32, tag="eps1")
        nc.vector.memset(eps1, 1e-5)
        warm = persist.tile([1, 1], F32, tag="warm")
        nc.scalar.activation(warm, eps1, AF.Ln)
        # app_rhs rows: 0 = nmr (-mu*rstd, filled by LN1), 1 = ones
        app_rhs = persist.tile([2, T], BF16, tag="apprhs")
        nc.vector.memset(app_rhs, 1.0)
        ones_T = persist.tile([1, T], BF16, tag="onesT")
        nc.vector.memset(ones_T, 1.0)
        heat_w = persist.tile([P, P], BF16, tag="heatw")
        nc.vector.memset(heat_w, 0.001)
        heat_src = persist.tile([P, TQ], BF16, tag="heatsrc")
        nc.vector.memset(heat_src, 0.001)

        xh_all = persist.tile([P, CJ, T], FP8, tag="xhall")
        xso = persist.tile([P, CJ, TQ], FP8, tag="xso")
        vdx = persist.tile([P, NT, H, P], FP8, tag="vdx")
        nc.vector.memset(vdx.rearrange("p t h (a d) -> p t h a d", a=2)[:, :, :, 0, :], SV)
        kTp = [persist.tile([P, T], BF16, tag=f"kTp{m}", name=f"kTp{m}")
               for m in range(NPAIR)]
        qTp = [persist.tile([P, TQ], BF16, tag=f"qTp{m}", name=f"qTp{m}")
               for m in range(NPAIR)]
        yT = persist.tile([P, CJ, TQ], FP8, tag="yT")
        x1 = [persist.tile([P, TQ], BF16, tag=f"x1{m}", name=f"x1{m}")
              for m in range(CJ)]
        xh2 = persist.tile([P, CJ, TQ], FP8, tag="xh2")
        h1 = persist.tile([P, FCJ, TQ], FP8, tag="h1")

        def own_view(t):
            """[P, NQT, P] even-block (own query) view of a [P, T] tile."""
            return t.rearrange("p (b c) -> p b c", c=P)[:, 0::2, :]

        # ================= phase A: LN1 =================
        with tc.tile_pool(name="ph_a", bufs=2) as ph_a, \
             tc.tile_pool(name="lna", bufs=1) as lna, \
             tc.tile_pool(name="ps_st", bufs=1, space="PSUM") as ps_st, \
             tc.tile_pool(name="ps_bc", bufs=1, space="PSUM") as ps_bc, \
             tc.tile_pool(name="ps_heat", bufs=1, space="PSUM") as ps_heat:
            heat_ps = ps_heat.tile([P, TQ], F32, tag="heat", name="heatA")

            def heat(n):
                for _ in range(n):
                    nc.tensor.matmul(heat_ps, heat_w, heat_src,
                                     start=True, stop=True,
                                     skip_group_check=True)
            heat(20)
            mu_ps = ps_st.tile([1, 2, TQ], F32, tag="mups", name="mups")
            sq_ps = ps_st.tile([1, 2, TQ], F32, tag="sqps", name="sqps")
            for m in range(CJ):
                b = xbf[m]
                s = ph_a.tile([P, T], BF16, tag="xsq")
                nc.vector.tensor_tensor(s, b, b, ALU.mult)
                for hf in range(2):
                    nc.tensor.matmul(mu_ps[:, hf, :], onesc_bf,
                                     b[:, hf * TQ:(hf + 1) * TQ],
                                     start=(m == 0), stop=(m == CJ - 1),
                                     skip_group_check=True)
                for hf in range(2):
                    nc.tensor.matmul(sq_ps[:, hf, :], onesc_bf,
                                     s[:, hf * TQ:(hf + 1) * TQ],
                                     start=(m == 0), stop=(m == CJ - 1),
                                     skip_group_check=True)
                heat(2)
            mu_f = lna.tile([1, T], F32, tag="muf")
            nc.vector.tensor_scalar(mu_f, mu_ps.rearrange("o a b -> o (a b)"),
                                    1.0 / C, None, ALU.mult)
            var_f = lna.tile([1, T], F32, tag="varf")
            nc.vector.tensor_scalar(var_f, sq_ps.rearrange("o a b -> o (a b)"),
                                    1.0 / C, None, ALU.mult)
            musq = lna.tile([1, T], F32, tag="musq")
            nc.vector.tensor_tensor(musq, mu_f, mu_f, ALU.mult)
            nc.vector.tensor_tensor(var_f, var_f, musq, ALU.subtract)
            # rstd = exp(-0.5*ln(var+eps)) keeps ACT on the ln/exp table set
            lv_f = lna.tile([1, T], F32, tag="lvf")
            nc.scalar.activation(lv_f, var_f, AF.Ln, bias=eps1)
            rstd_f = lna.tile([1, T], F32, tag="rstdf")
            nc.scalar.activation(rstd_f, lv_f, AF.Exp, scale=-0.5)
            nmr_f = lna.tile([1, T], F32, tag="nmrf")
            nc.vector.tensor_tensor(nmr_f, mu_f, rstd_f, ALU.mult)
            # app_rhs row 0 = -mu*rstd
            nc.vector.tensor_scalar(app_rhs[0:1, :], nmr_f, -1.0, None, ALU.mult)
            rstd_bf = lna.tile([1, T], BF16, tag="rstdbf")
            nc.vector.tensor_copy(rstd_bf, rstd_f)
            # broadcast rstd across partitions via K=1 matmuls
            rst_ps = ps_bc.tile([P, 2, TQ], F32, tag="rstps", name="rstps")
            for hf in range(2):
                nc.tensor.matmul(rst_ps[:, hf, :], ones_row,
                                 rstd_bf[:, hf * TQ:(hf + 1) * TQ],
                                 start=True, stop=True)
            heat(20)
            rst_s = lna.tile([P, T], BF16, tag="rsts")
            nc.vector.tensor_copy(rst_s, rst_ps.rearrange("p a b -> p (a b)"))
            # xs = x * rstd  (the -mu*rstd part is folded into each proj)
            for m in range(CJ):
                nc.vector.tensor_tensor(xh_all[:, m, :], xbf[m], rst_s,
                                        ALU.mult)
                heat(2)
        # own-query columns of xs, contiguous for the Q matmuls
        nc.gpsimd.dma_start(
            xso.rearrange("p c (b q) -> p c b q", q=P),
            xh_all.rearrange("p c (b q) -> p c b q", q=P)[:, :, 0::2, :])

        # ================= phase B: V =================
        with tc.tile_pool(name="ps_v", bufs=2, space="PSUM") as ps_v, \
             tc.tile_pool(name="ps_heatb", bufs=1, space="PSUM") as ps_heatb:
            heat_psb = ps_heatb.tile([P, TQ], F32, tag="heatb", name="heatB")
            for t in range(NT):
                for _ in range(2):
                    nc.tensor.matmul(heat_psb, heat_w, heat_src,
                                     start=True, stop=True,
                                     skip_group_check=True)
                pv = ps_v.tile([P, 2, TQ], F32, tag="pv")
                for k in range(3):
                    lhs = xh_all[:, 2 * k:2 * k + 2, t * P:(t + 1) * P]
                    for half in range(2):
                        nc.tensor.matmul(
                            pv[:, half, 0:384], lhs,
                            wv_t[:, 2 * k:2 * k + 2,
                                 half * 384:(half + 1) * 384],
                            start=(k == 0), stop=False, perf_mode=DR,
                            skip_group_check=True)
                for half in range(2):
                    nc.tensor.matmul(
                        pv[:, half, 0:384],
                        app_rhs[:, t * P:(t + 1) * P],
                        vapp_t[:, half * 384:(half + 1) * 384],
                        start=False, stop=True, skip_group_check=True)
                nc.vector.tensor_copy(
                    vdx.rearrange("p t (g s) (a d) -> p t g s a d",
                                  g=2, a=2)[:, t, :, :, 1, :],
                    pv[:, :, 0:384].rearrange("p a (h d) -> p a h d", d=HD))

        # ================= phase C: K, Q, attention (sw-pipelined) =========
        with tc.tile_pool(name="att", bufs=6) as att, \
             tc.tile_pool(name="ps_kq", bufs=2, space="PSUM") as ps_kq, \
             tc.tile_pool(name="ps_sc", bufs=2, space="PSUM") as ps_sc, \
             tc.tile_pool(name="ps_avd", bufs=2, space="PSUM") as ps_avd:

            def emit_kq(m):
                pks = [ps_kq.tile([P, TQ], F32, tag="kq", name=f"pk{m}_{tk}")
                       for tk in range(2)]
                for k in range(3):
                    for tk in range(2):
                        nc.tensor.matmul(
                            pks[tk], wk_t[:, 2 * k:2 * k + 2, m * P:(m + 1) * P],
                            xh_all[:, 2 * k:2 * k + 2, tk * TQ:(tk + 1) * TQ],
                            start=(k == 0), stop=False, perf_mode=DR,
                            skip_group_check=True)
                for tk in range(2):
                    nc.tensor.matmul(
                        pks[tk], kapp_t[:, m * P:(m + 1) * P],
                        app_rhs[:, tk * TQ:(tk + 1) * TQ],
                        start=False, stop=True, skip_group_check=True)
                    nc.scalar.activation(kTp[m][:, tk * TQ:(tk + 1) * TQ],
                                         pks[tk], AF.Identity)
                pq = ps_kq.tile([P, TQ], F32, tag="kq", name=f"pq{m}")
                for k in range(3):
                    nc.tensor.matmul(
                        pq, wq_t[:, 2 * k:2 * k + 2, m * P:(m + 1) * P],
                        xso[:, 2 * k:2 * k + 2, :],
                        start=(k == 0), stop=False, perf_mode=DR,
                        skip_group_check=True)
                nc.tensor.matmul(
                    pq, qapp_t[:, m * P:(m + 1) * P],
                    app_rhs.rearrange("r (b q) -> r b q", q=P)[:, 0::2, :],
                    start=False, stop=True, skip_group_check=True)
                nc.vector.tensor_copy(qTp[m], pq)

            def emit_scores(m, hh):
                hs = slice(hh * HD, (hh + 1) * HD)
                exs = []
                for j in range(2):
                    w = TQ - j * P
                    sc = ps_sc.tile([P, 2, TQ], F32, tag="sc")
                    for i in range(2):
                        nc.tensor.matmul(
                            sc[:, i, 0:w],
                            kTp[m][hs, (2 * j + i) * P:(2 * j + i + 1) * P],
                            qTp[m][hs, j * P:TQ],
                            start=True, stop=True, skip_group_check=True)
                    ex = att.tile([P, 2, TQ], FP8, tag="ex")
                    nc.scalar.activation(ex[:, :, 0:w], sc[:, :, 0:w],
                                         AF.Exp, scale=0.125 / (SQ * SQ))
                    eng = nc.vector if j == 0 else nc.gpsimd
                    eng.tensor_tensor(
                        ex[:, :, 0:P], ex[:, :, 0:P],
                        msk_t[:, 0:2 * P].rearrange("p (a b) -> p a b", a=2),
                        ALU.mult)
                    exs.append(ex)
                sc = ps_sc.tile([P, 2, TQ], F32, tag="sc")
                for i in range(2):
                    nc.tensor.matmul(
                        sc[:, 0, i * 256:i * 256 + 256],
                        kTp[m][hs, (4 + i) * P:(5 + i) * P],
                        qTp[m][hs, 2 * P:TQ],
                        start=True, stop=True, skip_group_check=True)
                    nc.tensor.matmul(
                        sc[:, 1, i * P:(i + 1) * P],
                        kTp[m][hs, (6 + i) * P:(7 + i) * P],
                        qTp[m][hs, 3 * P:TQ],
                        start=True, stop=True, skip_group_check=True)
                for _ in range(6):
                    nc.tensor.matmul(sc[:, 1, 2 * P:TQ], heat_w,
                                     heat_src[:, 0:2 * P],
                                     start=True, stop=True,
                                     skip_group_check=True)
                ex23 = att.tile([P, 2, TQ], FP8, tag="ex")
                nc.scalar.activation(ex23[:, 0, :], sc[:, 0, :],
                                     AF.Exp, scale=0.125 / (SQ * SQ))
                nc.scalar.activation(ex23[:, 1, 0:256], sc[:, 1, 0:256],
                                     AF.Exp, scale=0.125 / (SQ * SQ))
                nc.gpsimd.tensor_tensor(
                    ex23[:, 0, 0:384], ex23[:, 0, 0:384],
                    msk_t[:, 2 * P:2 * P + 384], ALU.mult)
                nc.vector.tensor_tensor(
                    ex23[:, 1, 0:256], ex23[:, 1, 0:256],
                    msk_t[:, 2 * P + 384:2 * P + 640], ALU.mult)
                exs.append(ex23)
                return exs

            def emit_avd(m, hh, exs):
                h = 2 * m + hh
                avd = ps_avd.tile([P, TQ], F32, tag="avd", name=f"avd{m}_{hh}")

                def av_rhs(j):
                    if j < 2:
                        return exs[j][:, :, 0:TQ - j * P]
                    if j == 2:
                        return exs[2].rearrange(
                            "p a (b c) -> p (a b) c", c=256)[:, 0:2, :]
                    return exs[2].rearrange(
                        "p a (b c) -> p (a b) c", c=P)[:, 4:6, :]
                for j in range(NQT):
                    nc.tensor.matmul(
                        avd[:, j * P:TQ], vdx[:, 2 * j:2 * j + 2, h, :],
                        av_rhs(j), start=(j == 0), stop=(j == NQT - 1),
                        perf_mode=DR, skip_group_check=True)
                if dbg:
                    sb_avd = att.tile([P, TQ], F32, tag="sbavd")
                    nc.vector.tensor_copy(sb_avd, avd)
                    nc.sync.dma_start(
                        dbg_avd[:, (2 * m + hh) * TQ:(2 * m + hh + 1) * TQ],
                        sb_avd)
                rd = att.tile([HD, TQ], F32, tag="rd")
                with nc.allow_low_precision(reason="softmax denom"):
                    nc.vector.reciprocal_approx_fast(rd, avd[0:HD, :])
                nc.vector.tensor_tensor(
                    yT[hh * HD:(hh + 1) * HD, m, :], avd[HD:P, :], rd,
                    ALU.mult)

            emit_kq(0)
            for m in range(NPAIR):
                r0 = emit_scores(m, 0)
                r1 = emit_scores(m, 1)
                if m + 1 < NPAIR:
                    emit_kq(m + 1)
                emit_avd(m, 0, r0)
                emit_avd(m, 1, r1)

        if dbg:
            nc.sync.dma_start(dbg_xh[:, :], xh_all.rearrange("p a b -> p (a b)"))
            nc.sync.dma_start(dbg_vdx[:, :], vdx.rearrange("p a b c -> p (a b c)"))
            for m in range(NPAIR):
                nc.sync.dma_start(dbg_k[m * P:(m + 1) * P, :], kTp[m])
                nc.sync.dma_start(dbg_q[m * P:(m + 1) * P, :], qTp[m])
            nc.sync.dma_start(dbg_yT[:, :], yT.rearrange("p a b -> p (a b)"))

        # ================= phase D: Wo + residual =================
        with tc.tile_pool(name="ps_wo", bufs=3, space="PSUM") as ps_wo, \
             tc.tile_pool(name="ps_heatd", bufs=1, space="PSUM") as ps_heatd:
            heat_ps2 = ps_heatd.tile([P, TQ], F32, tag="heatd", name="heatD")

            def heat2(n):
                for _ in range(n):
                    nc.tensor.matmul(heat_ps2, heat_w, heat_src,
                                     start=True, stop=True,
                                     skip_group_check=True)
            for mo in range(CJ):
                pm = ps_wo.tile([P, TQ], F32, tag="wops")
                for k in range(3):
                    nc.tensor.matmul(
                        pm, wo_t[:, 2 * k:2 * k + 2, mo * P:(mo + 1) * P],
                        yT[:, 2 * k:2 * k + 2, :],
                        start=(k == 0), stop=False, perf_mode=DR,
                        skip_group_check=True)
                nc.tensor.matmul(pm, woapp_t[:, mo * P:(mo + 1) * P],
                                 ones_T[:, 0:TQ],
                                 start=False, stop=True, skip_group_check=True)
                nc.vector.scalar_tensor_tensor(
                    x1[mo], pm, 1.0 / SO, own_view(xbf[mo]),
                    ALU.mult, ALU.add)
                heat2(3)

        if dbg:
            for m in range(CJ):
                nc.sync.dma_start(dbg_x1[m * P:(m + 1) * P, :], x1[m])

        # ================= phase E: LN2 =================
        with tc.tile_pool(name="ph_e", bufs=2) as ph_e, \
             tc.tile_pool(name="lne", bufs=1) as lne, \
             tc.tile_pool(name="ps_st2", bufs=1, space="PSUM") as ps_st2, \
             tc.tile_pool(name="ps_bc2", bufs=1, space="PSUM") as ps_bc2, \
             tc.tile_pool(name="ps_heate", bufs=1, space="PSUM") as ps_heate:
            heat_ps3 = ps_heate.tile([P, TQ], F32, tag="heate", name="heatE")

            def heat3(n):
                for _ in range(n):
                    nc.tensor.matmul(heat_ps3, heat_w, heat_src,
                                     start=True, stop=True,
                                     skip_group_check=True)
            mu_ps = ps_st2.tile([1, TQ], F32, tag="mups2", name="mups2")
            sq_ps = ps_st2.tile([1, TQ], F32, tag="sqps2", name="sqps2")
            for m in range(CJ):
                s = ph_e.tile([P, TQ], BF16, tag="x1sq")
                nc.vector.tensor_tensor(s, x1[m], x1[m], ALU.mult)
                nc.tensor.matmul(mu_ps, onesc_bf, x1[m],
                                 start=(m == 0), stop=(m == CJ - 1),
                                 skip_group_check=True)
                nc.tensor.matmul(sq_ps, onesc_bf, s,
                                 start=(m == 0), stop=(m == CJ - 1),
                                 skip_group_check=True)
                heat3(2)
            mu_f = lne.tile([1, TQ], F32, tag="muf2")
            nc.vector.tensor_scalar(mu_f, mu_ps, 1.0 / C, None, ALU.mult)
            var_f = lne.tile([1, TQ], F32, tag="varf2")
            nc.vector.tensor_scalar(var_f, sq_ps, 1.0 / C, None, ALU.mult)
            musq = lne.tile([1, TQ], F32, tag="musq2")
            nc.vector.tensor_tensor(musq, mu_f, mu_f, ALU.mult)
            nc.vector.tensor_tensor(var_f, var_f, musq, ALU.subtract)
            lv_f = lne.tile([1, TQ], F32, tag="lvf2")
            nc.scalar.activation(lv_f, var_f, AF.Ln, bias=eps1)
            rstd_f = lne.tile([1, TQ], F32, tag="rstdf2")
            nc.scalar.activation(rstd_f, lv_f, AF.Exp, scale=-0.5)
            nmr_f = lne.tile([1, TQ], F32, tag="nmrf2")
            nc.vector.tensor_tensor(nmr_f, mu_f, rstd_f, ALU.mult)
            rstd_bf = lne.tile([1, TQ], BF16, tag="rstdbf2")
            nc.vector.tensor_copy(rstd_bf, rstd_f)
            nmr_bf = lne.tile([1, TQ], BF16, tag="nmrbf2")
            nc.vector.tensor_copy(nmr_bf, nmr_f)
            rst_ps = ps_bc2.tile([P, TQ], F32, tag="rstps2", name="rstps2")
            nrm_ps = ps_bc2.tile([P, TQ], F32, tag="nrmps2", name="nrmps2")
            heat3(20)
            nc.tensor.matmul(rst_ps, ones_row, rstd_bf, start=True, stop=True)
            nc.tensor.matmul(nrm_ps, mones_row, nmr_bf, start=True, stop=True)
            heat3(12)
            rst_s = lne.tile([P, TQ], BF16, tag="rsts2")
            nc.vector.tensor_copy(rst_s, rst_ps)
            nrm_s = lne.tile([P, TQ], BF16, tag="nrms2")
            nc.vector.tensor_copy(nrm_s, nrm_ps)
            for m in range(CJ):
                tmp = ph_e.tile([P, TQ], BF16, tag="xnt2")
                nc.vector.tensor_tensor(tmp, x1[m], rst_s, ALU.mult)
                nc.vector.tensor_tensor(xh2[:, m, :], tmp, nrm_s, ALU.add)

        if dbg:
            nc.sync.dma_start(dbg_xh2[:, :], xh2.rearrange("p a b -> p (a b)"))

        # ================= phases F+G: FC + gelu + proj =================
        with tc.tile_pool(name="ph_f", bufs=8) as ph_f, \
             tc.tile_pool(name="ph_g", bufs=2) as ph_g, \
             tc.tile_pool(name="ps_fc", bufs=2, space="PSUM") as ps_fc, \
             tc.tile_pool(name="ps_pj", bufs=1, space="PSUM") as ps_pj:
            pj = ps_pj.tile([P, CJ, TQ], F32, tag="pj", name="pj")
            BLK = 6
            for b0 in range(0, FCJ, BLK):
                xbs, s1s = {}, {}
                for mo in range(b0, b0 + BLK):
                    pf = ps_fc.tile([P, TQ], F32, tag="fc")
                    for k in range(3):
                        nc.tensor.matmul(
                            pf, wfc_t[:, 2 * k:2 * k + 2, mo * P:(mo + 1) * P],
                            xh2[:, 2 * k:2 * k + 2, :],
                            start=(k == 0), stop=(k == 2), perf_mode=DR,
                            skip_group_check=True)
                    xb = ph_f.tile([P, TQ], BF16, tag="xb")
                    nc.vector.tensor_scalar(xb, pf, 1.0 / SF,
                                            bfcb_t[:, mo:mo + 1],
                                            ALU.mult, ALU.add)
                    s1 = ph_f.tile([P, TQ], BF16, tag="s1")
                    nc.scalar.activation(s1, xb, AF.Square)
                    xbs[mo], s1s[mo] = xb, s1
                for mo in range(b0, b0 + BLK):
                    s2 = ph_f.tile([P, TQ], BF16, tag="s2")
                    nc.gpsimd.tensor_tensor(s2, s1s[mo], s1s[mo], ALU.mult)
                    u = ph_f.tile([P, TQ], BF16, tag="u")
                    nc.scalar.activation(u, s2, AF.Sigmoid, scale=2.0 * GA2)
                    h1_eng = nc.gpsimd if mo % 3 == 2 else nc.vector
                    h1_eng.tensor_tensor(h1[:, mo, :], xbs[mo], u, ALU.mult)
                    if mo % 2 == 1:
                        r = mo // 2
                        for c in range(CJ):
                            nc.tensor.matmul(
                                pj[:, c, :],
                                wpj_t[:, 2 * r:2 * r + 2, c * P:(c + 1) * P],
                                h1[:, 2 * r:2 * r + 2, :],
                                start=(r == 0), stop=False, perf_mode=DR,
                                skip_group_check=True)
            for c in range(CJ):
                nc.tensor.matmul(pj[:, c, :], pjapp_t[:, c * P:(c + 1) * P],
                                 ones_T[:, 0:TQ],
                                 start=False, stop=True, skip_group_check=True)
                oj = ph_g.tile([P, TQ], F32, tag="oj")
                nc.vector.scalar_tensor_tensor(
                    oj, pj[:, c, :], 1.0 / SP, x1[c],
                    ALU.mult, ALU.add)
                nc.sync.dma_start(out_d[c * P:(c + 1) * P, :], oj)

    nc.compile()
    return nc


def _get_nc():
    if "nc" not in _CACHED:
        _CACHED["nc"] = _build_nc()
    return _CACHED["nc"]


def _perm_blocks(p):
    return [p, 1 - p, 2 + p, 3 - p, 4 + p, 5 - p, 6 + p, 7 - p]


def _fp8(a):
    return np.clip(np.asarray(a, np.float32), -240.0, 240.0).astype(
        ml_dtypes.float8_e4m3)


def _build_in_maps(x, ln1_scale, ln1_bias, Wqkv, bqkv, Wo, bo,
                   ln2_scale, ln2_bias, Wfc, bfc, Wproj, bproj):
    bf16 = ml_dtypes.bfloat16
    x = np.asarray(x, np.float32)
    # Fold LN scale/bias into the following projection (exact):
    Wq64 = np.asarray(ln1_scale, np.float64)[:, None] * np.asarray(Wqkv, np.float64)
    bq64 = np.asarray(bqkv, np.float64) + np.asarray(ln1_bias, np.float64) @ Wq64
    Wfc64 = np.asarray(ln2_scale, np.float64)[:, None] * np.asarray(Wfc, np.float64)
    bfc64 = np.asarray(bfc, np.float64) + np.asarray(ln2_bias, np.float64) @ Wfc64
    colmap = np.arange(3 * C).reshape(H, 3, HD)
    Wq64 = Wq64.astype(np.float32)
    bq64 = bq64.astype(np.float32)

    def pcm(w, scale):  # [C, n] -> fp8 [128, CJ, n], scaled
        n = w.shape[1]
        return np.ascontiguousarray(
            _fp8(np.asarray(w, np.float32).reshape(CJ, P, n)
                 .transpose(1, 0, 2) * scale))

    wq_h = Wq64[:, colmap[:, 0, :].ravel()]
    wk_h = Wq64[:, colmap[:, 1, :].ravel()]
    wv_h = Wq64[:, colmap[:, 2, :].ravel()]
    bq_h = bq64[colmap[:, 0, :].ravel()]
    bk_h = bq64[colmap[:, 1, :].ravel()]
    bv_h = bq64[colmap[:, 2, :].ravel()]

    wq8 = pcm(wq_h, SQ)
    wk8 = pcm(wk_h, SQ)
    wv8 = pcm(wv_h, SV)
    wo8 = pcm(np.asarray(Wo, np.float32), SO)
    wfc8 = pcm(Wfc64.astype(np.float32), SF)
    wpj8 = np.ascontiguousarray(
        _fp8(np.asarray(Wproj, np.float32).reshape(FCJ, P, C)
             .transpose(1, 0, 2) * SP))

    def app2(w8, bias, bscale):
        cs = w8.astype(np.float32).sum((0, 1))      # colsum of scaled fp8 w
        return np.ascontiguousarray(
            np.stack([cs, np.asarray(bias, np.float32) * bscale]).astype(bf16))

    shared = {
        "wv": wv8, "wk": wk8, "wq": wq8, "wo": wo8, "wfc": wfc8, "wpj": wpj8,
        "kapp": app2(wk8, bk_h, SQ),
        "qapp": app2(wq8, bq_h, SQ),
        "vapp": app2(wv8, bv_h, SV),
        "woapp": np.ascontiguousarray(
            (np.asarray(bo, np.float32) * SO)[None, :].astype(bf16)),
        "pjapp": np.ascontiguousarray(
            (np.asarray(bproj, np.float32) * SP)[None, :].astype(bf16)),
        "bfcb": np.ascontiguousarray(
            bfc64.astype(np.float32).reshape(FCJ, P).T),
    }
    in_maps = []
    own_toks = []
    for cidx in range(N_CORES):
        s, p = divmod(cidx, 2)
        blocks = _perm_blocks(p)
        tok = np.concatenate([np.arange(b * P, (b + 1) * P) for b in blocks])
        own = np.concatenate([np.arange(b * P, (b + 1) * P)
                              for b in blocks[0::2]])
        own_toks.append((s, own))
        fp8 = ml_dtypes.float8_e4m3
        tril = (np.arange(P)[None, :] >= np.arange(P)[:, None]).astype(np.float32)
        fl = np.full((P, P), float(p), np.float32)
        on = np.ones((P, P), np.float32)
        msk = np.concatenate([tril, fl, tril, on, fl, tril, fl], axis=1)
        in_maps.append({
            "xt": np.ascontiguousarray(x[s][tok].T.astype(bf16)),
            "msk": np.ascontiguousarray(msk.astype(fp8)),
            **shared,
        })
    return in_maps, own_toks


def kernel(x, ln1_scale, ln1_bias, Wqkv, bqkv, Wo, bo,
           ln2_scale, ln2_bias, Wfc, bfc, Wproj, bproj):
    from concourse.bass_utils import run_bass_kernel_spmd

    in_maps, own_toks = _build_in_maps(
        x, ln1_scale, ln1_bias, Wqkv, bqkv, Wo, bo,
        ln2_scale, ln2_bias, Wfc, bfc, Wproj, bproj)
    nc = _get_nc()
    res = run_bass_kernel_spmd(nc, in_maps, list(range(N_CORES)))

    out = np.empty((B, T, C), np.float32)
    for cidx in range(N_CORES):
        s, own = own_toks[cidx]
        out[s][own] = res.results[cidx]["out"].T
    return out


# revision 31
# speedup vs baseline: 1.0264x; 1.0264x over previous
"""Trainium2 Bass kernel for a GPT-2-style transformer block (v3, fp8).

B=4, T=1024, C=768, H=12 heads (HD=64). 8 NeuronCores.
2 cores per sequence; each core owns 512 query tokens (block-permuted).

v3 vs v2 (bf16 baseline):
- All big matmuls fp8e4 DoubleRow (contraction in 256-pairs): ~1.84x PE.
- Weights host-scaled into fp8 range (x64/x128/x16); scales folded into
  the exp scale, sigmoid scale, and epilogue scalar ops. No unscale ops.
- LN applied as xs = x*rstd only; the -mu*rstd and bias terms enter each
  projection as a K=2 appended matmul (wsum (x) nmr + bias (x) ones).
- Causal masks folded into the score PSUM via appended matmuls
  (tril-bias x identity; per-core flag row x ones) -> no DVE mask work.
- Softmax denominators via fp8 ones-matmuls into a shared [33,TQ] PSUM;
  reciprocal row broadcast to 128 partitions via SBUF->SBUF DMA.
- gelu: xb on DVE, squares on gpsimd, sigmoid on ACT, h1 on DVE.
- proj matmuls interleaved into the FC loop (6-bank persistent PSUM).
"""

import numpy as np
import ml_dtypes

P = 128
B, T, C, H = 4, 1024, 768, 12
HD = C // H        # 64
CJ = C // P        # 6 chunks
NT = T // P        # 8 token blocks
TQ = 512           # own query tokens per core
NQT = TQ // P      # 4 q blocks
FC = 4 * C         # 3072
FCJ = FC // P      # 24
NPAIR = H // 2     # 6 head pairs
GA2 = 0.035677408136300527   # sqrt(2/pi)*0.044715
N_CORES = 8

SQ = 64.0     # q,k weight scale
SV = 16.0     # v weight scale
SO = 128.0    # Wo scale
SF = 64.0     # Wfc scale
SP = 128.0    # Wproj scale
MASKNEG = -1e9

_CACHED = {}


def _build_nc():
    import concourse.bass as bass
    from concourse import bacc, mybir
    import concourse.tile as tile
    from contextlib import ExitStack

    F32 = mybir.dt.float32
    BF16 = mybir.dt.bfloat16
    FP8 = mybir.dt.float8e4
    AF = mybir.ActivationFunctionType
    ALU = mybir.AluOpType
    DR = mybir.MatmulPerfMode.DoubleRow

    nc = bacc.Bacc()

    xt_d = nc.declare_dram_parameter("xt", [C, T], BF16, isOutput=False)
    wv_d = nc.declare_dram_parameter("wv", [P, CJ, C], FP8, isOutput=False)
    wk_d = nc.declare_dram_parameter("wk", [P, CJ, C], FP8, isOutput=False)
    wq_d = nc.declare_dram_parameter("wq", [P, CJ, C], FP8, isOutput=False)
    wo_d = nc.declare_dram_parameter("wo", [P, CJ, C], FP8, isOutput=False)
    wfc_d = nc.declare_dram_parameter("wfc", [P, CJ, FC], FP8, isOutput=False)
    wpj_d = nc.declare_dram_parameter("wpj", [P, FCJ, C], FP8, isOutput=False)
    kapp_d = nc.declare_dram_parameter("kapp", [2, C], BF16, isOutput=False)
    qapp_d = nc.declare_dram_parameter("qapp", [2, C], BF16, isOutput=False)
    vapp_d = nc.declare_dram_parameter("vapp", [2, C], BF16, isOutput=False)
    woapp_d = nc.declare_dram_parameter("woapp", [1, C], BF16, isOutput=False)
    pjapp_d = nc.declare_dram_parameter("pjapp", [1, C], BF16, isOutput=False)
    bfcb_d = nc.declare_dram_parameter("bfcb", [P, FCJ], F32, isOutput=False)
    msk_d = nc.declare_dram_parameter("msk", [P, 2 * P + 384 + 256], FP8, isOutput=False)
    out_d = nc.declare_dram_parameter("out", [C, TQ], F32, isOutput=True)
    import os
    dbg = os.environ.get("KV3_DEBUG") == "1"
    if dbg:
        dbg_xh = nc.declare_dram_parameter("dbg_xh", [P, CJ * T], FP8, isOutput=True)
        dbg_k = nc.declare_dram_parameter("dbg_k", [NPAIR * P, T], BF16, isOutput=True)
        dbg_q = nc.declare_dram_parameter("dbg_q", [NPAIR * P, TQ], BF16, isOutput=True)
        dbg_vdx = nc.declare_dram_parameter("dbg_vdx", [P, NT * H * P], FP8, isOutput=True)
        dbg_yT = nc.declare_dram_parameter("dbg_yT", [P, CJ * TQ], FP8, isOutput=True)
        dbg_x1 = nc.declare_dram_parameter("dbg_x1", [CJ * P, TQ], BF16, isOutput=True)
        dbg_xh2 = nc.declare_dram_parameter("dbg_xh2", [P, CJ * TQ], FP8, isOutput=True)
        dbg_h1 = nc.declare_dram_parameter("dbg_h1", [P, FCJ * TQ], FP8, isOutput=True)
        dbg_avd = nc.declare_dram_parameter("dbg_avd", [P, NPAIR * 2 * TQ], F32, isOutput=True)
        dbg_rd = nc.declare_dram_parameter("dbg_rd", [HD, NPAIR * 2 * TQ], F32, isOutput=True)

    with tile.TileContext(nc) as tc, ExitStack() as ctx:
        persist = ctx.enter_context(tc.tile_pool(name="persist", bufs=1))

        # ---------- DMAs spread across engine queues ----------
        xbf = [persist.tile([P, T], BF16, tag=f"xbf{m}", name=f"xbf{m}")
               for m in range(CJ)]
        for m in range(CJ):
            eng = nc.sync if m % 2 == 0 else nc.scalar
            eng.dma_start(xbf[m], xt_d[m * P:(m + 1) * P, :])
        wv_t = persist.tile([P, CJ, C], FP8, tag="wv")
        nc.sync.dma_start(wv_t, wv_d[:, :, :])
        wk_t = persist.tile([P, CJ, C], FP8, tag="wk")
        nc.sync.dma_start(wk_t, wk_d[:, :, :])
        wq_t = persist.tile([P, CJ, C], FP8, tag="wq")
        nc.scalar.dma_start(wq_t, wq_d[:, :, :])
        wo_t = persist.tile([P, CJ, C], FP8, tag="wo")
        nc.scalar.dma_start(wo_t, wo_d[:, :, :])
        wfc_t = persist.tile([P, CJ, FC], FP8, tag="wfc")
        nc.scalar.dma_start(wfc_t, wfc_d[:, :, :])
        wpj_t = persist.tile([P, FCJ, C], FP8, tag="wpj")
        nc.gpsimd.dma_start(wpj_t, wpj_d[:, :, :])
        kapp_t = persist.tile([2, C], BF16, tag="kapp")
        nc.scalar.dma_start(kapp_t, kapp_d[:, :])
        qapp_t = persist.tile([2, C], BF16, tag="qapp")
        nc.scalar.dma_start(qapp_t, qapp_d[:, :])
        vapp_t = persist.tile([2, C], BF16, tag="vapp")
        nc.scalar.dma_start(vapp_t, vapp_d[:, :])
        woapp_t = persist.tile([1, C], BF16, tag="woapp")
        nc.scalar.dma_start(woapp_t, woapp_d[:, :])
        pjapp_t = persist.tile([1, C], BF16, tag="pjapp")
        nc.scalar.dma_start(pjapp_t, pjapp_d[:, :])
        bfcb_t = persist.tile([P, FCJ], F32, tag="bfcb")
        nc.scalar.dma_start(bfcb_t, bfcb_d[:, :])
        msk_t = persist.tile([P, 2 * P + 384 + 256], FP8, tag="msk")
        nc.scalar.dma_start(msk_t, msk_d[:, :])

        # ---------- constants ----------
        onesc_bf = persist.tile([P, 1], BF16, tag="onescbf")
        nc.vector.memset(onesc_bf, 1.0)
        ones_row = persist.tile([1, P], BF16, tag="onesrow")
        nc.vector.memset(ones_row, 1.0)
        mones_row = persist.tile([1, P], BF16, tag="monesrow")
        nc.vector.memset(mones_row, -1.0)
        ones_hd = persist.tile([1, HD], BF16, tag="oneshd")
        nc.vector.memset(ones_hd, 1.0)
        eps1 = persist.tile([1, 1], F32, tag="eps1")
        nc.vector.memset(eps1, 1e-5)
        warm = persist.tile([1, 1], F32, tag="warm")
        nc.scalar.activation(warm, eps1, AF.Ln)
        # app_rhs rows: 0 = nmr (-mu*rstd, filled by LN1), 1 = ones
        app_rhs = persist.tile([2, T], BF16, tag="apprhs")
        nc.vector.memset(app_rhs, 1.0)
        ones_T = persist.tile([1, T], BF16, tag="onesT")
        nc.vector.memset(ones_T, 1.0)
        heat_w = persist.tile([P, P], BF16, tag="heatw")
        nc.vector.memset(heat_w, 0.001)
        heat_src = persist.tile([P, TQ], BF16, tag="heatsrc")
        nc.vector.memset(heat_src, 0.001)

        xh_all = persist.tile([P, CJ, T], FP8, tag="xhall")
        xso = persist.tile([P, CJ, TQ], FP8, tag="xso")
        vdx = persist.tile([P, NT, H, P], FP8, tag="vdx")
        nc.vector.memset(vdx.rearrange("p t h (a d) -> p t h a d", a=2)[:, :, :, 0, :], SV)
        kTp = [persist.tile([P, T], BF16, tag=f"kTp{m}", name=f"kTp{m}")
               for m in range(NPAIR)]
        qTp = [persist.tile([P, TQ], BF16, tag=f"qTp{m}", name=f"qTp{m}")
               for m in range(NPAIR)]
        yT = persist.tile([P, CJ, TQ], FP8, tag="yT")
        x1 = [persist.tile([P, TQ], BF16, tag=f"x1{m}", name=f"x1{m}")
              for m in range(CJ)]
        xh2 = persist.tile([P, CJ, TQ], FP8, tag="xh2")
        h1 = persist.tile([P, FCJ, TQ], FP8, tag="h1")

        def own_view(t):
            """[P, NQT, P] even-block (own query) view of a [P, T] tile."""
            return t.rearrange("p (b c) -> p b c", c=P)[:, 0::2, :]

        # ================= phase A: LN1 =================
        with tc.tile_pool(name="ph_a", bufs=2) as ph_a, \
             tc.tile_pool(name="lna", bufs=1) as lna, \
             tc.tile_pool(name="ps_st", bufs=1, space="PSUM") as ps_st, \
             tc.tile_pool(name="ps_bc", bufs=1, space="PSUM") as ps_bc, \
             tc.tile_pool(name="ps_heat", bufs=1, space="PSUM") as ps_heat:
            heat_ps = ps_heat.tile([P, TQ], F32, tag="heat", name="heatA")

            def heat(n):
                for _ in range(n):
                    nc.tensor.matmul(heat_ps, heat_w, heat_src,
                                     start=True, stop=True,
                                     skip_group_check=True)
            heat(20)
            mu_ps = ps_st.tile([1, 2, TQ], F32, tag="mups", name="mups")
            sq_ps = ps_st.tile([1, 2, TQ], F32, tag="sqps", name="sqps")
            for m in range(CJ):
                b = xbf[m]
                s = ph_a.tile([P, T], BF16, tag="xsq")
                nc.vector.tensor_tensor(s, b, b, ALU.mult)
                for hf in range(2):
                    nc.tensor.matmul(mu_ps[:, hf, :], onesc_bf,
                                     b[:, hf * TQ:(hf + 1) * TQ],
                                     start=(m == 0), stop=(m == CJ - 1),
                                     skip_group_check=True)
                for hf in range(2):
                    nc.tensor.matmul(sq_ps[:, hf, :], onesc_bf,
                                     s[:, hf * TQ:(hf + 1) * TQ],
                                     start=(m == 0), stop=(m == CJ - 1),
                                     skip_group_check=True)
                heat(2)
            mu_f = lna.tile([1, T], F32, tag="muf")
            nc.vector.tensor_scalar(mu_f, mu_ps.rearrange("o a b -> o (a b)"),
                                    1.0 / C, None, ALU.mult)
            var_f = lna.tile([1, T], F32, tag="varf")
            nc.vector.tensor_scalar(var_f, sq_ps.rearrange("o a b -> o (a b)"),
                                    1.0 / C, None, ALU.mult)
            musq = lna.tile([1, T], F32, tag="musq")
            nc.vector.tensor_tensor(musq, mu_f, mu_f, ALU.mult)
            nc.vector.tensor_tensor(var_f, var_f, musq, ALU.subtract)
            # rstd = exp(-0.5*ln(var+eps)) keeps ACT on the ln/exp table set
            lv_f = lna.tile([1, T], F32, tag="lvf")
            nc.scalar.activation(lv_f, var_f, AF.Ln, bias=eps1)
            rstd_f = lna.tile([1, T], F32, tag="rstdf")
            nc.scalar.activation(rstd_f, lv_f, AF.Exp, scale=-0.5)
            nmr_f = lna.tile([1, T], F32, tag="nmrf")
            nc.vector.tensor_tensor(nmr_f, mu_f, rstd_f, ALU.mult)
            # app_rhs row 0 = -mu*rstd
            nc.vector.tensor_scalar(app_rhs[0:1, :], nmr_f, -1.0, None, ALU.mult)
            rstd_bf = lna.tile([1, T], BF16, tag="rstdbf")
            nc.vector.tensor_copy(rstd_bf, rstd_f)
            # broadcast rstd across partitions via K=1 matmuls
            rst_ps = ps_bc.tile([P, 2, TQ], F32, tag="rstps", name="rstps")
            for hf in range(2):
                nc.tensor.matmul(rst_ps[:, hf, :], ones_row,
                                 rstd_bf[:, hf * TQ:(hf + 1) * TQ],
                                 start=True, stop=True)
            heat(20)
            rst_s = lna.tile([P, T], BF16, tag="rsts")
            nc.vector.tensor_copy(rst_s, rst_ps.rearrange("p a b -> p (a b)"))
            # xs = x * rstd  (the -mu*rstd part is folded into each proj)
            for m in range(CJ):
                nc.vector.tensor_tensor(xh_all[:, m, :], xbf[m], rst_s,
                                        ALU.mult)
                heat(2)
        # own-query columns of xs, contiguous for the Q matmuls
        nc.gpsimd.dma_start(
            xso.rearrange("p c (b q) -> p c b q", q=P),
            xh_all.rearrange("p c (b q) -> p c b q", q=P)[:, :, 0::2, :])

        # ================= phase B: V =================
        with tc.tile_pool(name="ps_v", bufs=2, space="PSUM") as ps_v, \
             tc.tile_pool(name="ps_heatb", bufs=1, space="PSUM") as ps_heatb:
            heat_psb = ps_heatb.tile([P, TQ], F32, tag="heatb", name="heatB")
            for t in range(NT):
                for _ in range(2):
                    nc.tensor.matmul(heat_psb, heat_w, heat_src,
                                     start=True, stop=True,
                                     skip_group_check=True)
                pv = ps_v.tile([P, 2, TQ], F32, tag="pv")
                for k in range(3):
                    lhs = xh_all[:, 2 * k:2 * k + 2, t * P:(t + 1) * P]
                    for half in range(2):
                        nc.tensor.matmul(
                            pv[:, half, 0:384], lhs,
                            wv_t[:, 2 * k:2 * k + 2,
                                 half * 384:(half + 1) * 384],
                            start=(k == 0), stop=False, perf_mode=DR,
                            skip_group_check=True)
                for half in range(2):
                    nc.tensor.matmul(
                        pv[:, half, 0:384],
                        app_rhs[:, t * P:(t + 1) * P],
                        vapp_t[:, half * 384:(half + 1) * 384],
                        start=False, stop=True, skip_group_check=True)
                nc.vector.tensor_copy(
                    vdx.rearrange("p t (g s) (a d) -> p t g s a d",
                                  g=2, a=2)[:, t, :, :, 1, :],
                    pv[:, :, 0:384].rearrange("p a (h d) -> p a h d", d=HD))

        # ================= phase C: K, Q, attention (sw-pipelined) =========
        with tc.tile_pool(name="att", bufs=6) as att, \
             tc.tile_pool(name="ps_kq", bufs=2, space="PSUM") as ps_kq, \
             tc.tile_pool(name="ps_sc", bufs=2, space="PSUM") as ps_sc, \
             tc.tile_pool(name="ps_avd", bufs=2, space="PSUM") as ps_avd:

            def emit_kq(m):
                pks = [ps_kq.tile([P, TQ], F32, tag="kq", name=f"pk{m}_{tk}")
                       for tk in range(2)]
                for k in range(3):
                    for tk in range(2):
                        nc.tensor.matmul(
                            pks[tk], wk_t[:, 2 * k:2 * k + 2, m * P:(m + 1) * P],
                            xh_all[:, 2 * k:2 * k + 2, tk * TQ:(tk + 1) * TQ],
                            start=(k == 0), stop=False, perf_mode=DR,
                            skip_group_check=True)
                for tk in range(2):
                    nc.tensor.matmul(
                        pks[tk], kapp_t[:, m * P:(m + 1) * P],
                        app_rhs[:, tk * TQ:(tk + 1) * TQ],
                        start=False, stop=True, skip_group_check=True)
                    nc.vector.tensor_copy(kTp[m][:, tk * TQ:(tk + 1) * TQ],
                                          pks[tk])
                pq = ps_kq.tile([P, TQ], F32, tag="kq", name=f"pq{m}")
                for k in range(3):
                    nc.tensor.matmul(
                        pq, wq_t[:, 2 * k:2 * k + 2, m * P:(m + 1) * P],
                        xso[:, 2 * k:2 * k + 2, :],
                        start=(k == 0), stop=False, perf_mode=DR,
                        skip_group_check=True)
                nc.tensor.matmul(
                    pq, qapp_t[:, m * P:(m + 1) * P],
                    app_rhs.rearrange("r (b q) -> r b q", q=P)[:, 0::2, :],
                    start=False, stop=True, skip_group_check=True)
                nc.vector.tensor_copy(qTp[m], pq)

            def emit_scores(m, hh):
                hs = slice(hh * HD, (hh + 1) * HD)
                exs = []
                for j in range(2):
                    w = TQ - j * P
                    sc = ps_sc.tile([P, 2, TQ], F32, tag="sc")
                    for i in range(2):
                        nc.tensor.matmul(
                            sc[:, i, 0:w],
                            kTp[m][hs, (2 * j + i) * P:(2 * j + i + 1) * P],
                            qTp[m][hs, j * P:TQ],
                            start=True, stop=True, skip_group_check=True)
                    ex = att.tile([P, 2, TQ], FP8, tag="ex")
                    nc.scalar.activation(ex[:, :, 0:w], sc[:, :, 0:w],
                                         AF.Exp, scale=0.125 / (SQ * SQ))
                    eng = nc.vector if j == 0 else nc.gpsimd
                    eng.tensor_tensor(
                        ex[:, :, 0:P], ex[:, :, 0:P],
                        msk_t[:, 0:2 * P].rearrange("p (a b) -> p a b", a=2),
                        ALU.mult)
                    exs.append(ex)
                sc = ps_sc.tile([P, 2, TQ], F32, tag="sc")
                for i in range(2):
                    nc.tensor.matmul(
                        sc[:, 0, i * 256:i * 256 + 256],
                        kTp[m][hs, (4 + i) * P:(5 + i) * P],
                        qTp[m][hs, 2 * P:TQ],
                        start=True, stop=True, skip_group_check=True)
                    nc.tensor.matmul(
                        sc[:, 1, i * P:(i + 1) * P],
                        kTp[m][hs, (6 + i) * P:(7 + i) * P],
                        qTp[m][hs, 3 * P:TQ],
                        start=True, stop=True, skip_group_check=True)
                for _ in range(6):
                    nc.tensor.matmul(sc[:, 1, 2 * P:TQ], heat_w,
                                     heat_src[:, 0:2 * P],
                                     start=True, stop=True,
                                     skip_group_check=True)
                ex23 = att.tile([P, 2, TQ], FP8, tag="ex")
                nc.scalar.activation(ex23[:, 0, :], sc[:, 0, :],
                                     AF.Exp, scale=0.125 / (SQ * SQ))
                nc.scalar.activation(ex23[:, 1, 0:256], sc[:, 1, 0:256],
                                     AF.Exp, scale=0.125 / (SQ * SQ))
                nc.gpsimd.tensor_tensor(
                    ex23[:, 0, 0:384], ex23[:, 0, 0:384],
                    msk_t[:, 2 * P:2 * P + 384], ALU.mult)
                nc.vector.tensor_tensor(
                    ex23[:, 1, 0:256], ex23[:, 1, 0:256],
                    msk_t[:, 2 * P + 384:2 * P + 640], ALU.mult)
                exs.append(ex23)
                return exs

            def emit_avd(m, hh, exs):
                h = 2 * m + hh
                avd = ps_avd.tile([P, TQ], F32, tag="avd", name=f"avd{m}_{hh}")

                def av_rhs(j):
                    if j < 2:
                        return exs[j][:, :, 0:TQ - j * P]
                    if j == 2:
                        return exs[2].rearrange(
                            "p a (b c) -> p (a b) c", c=256)[:, 0:2, :]
                    return exs[2].rearrange(
                        "p a (b c) -> p (a b) c", c=P)[:, 4:6, :]
                for j in range(NQT):
                    nc.tensor.matmul(
                        avd[:, j * P:TQ], vdx[:, 2 * j:2 * j + 2, h, :],
                        av_rhs(j), start=(j == 0), stop=(j == NQT - 1),
                        perf_mode=DR, skip_group_check=True)
                if dbg:
                    sb_avd = att.tile([P, TQ], F32, tag="sbavd")
                    nc.vector.tensor_copy(sb_avd, avd)
                    nc.sync.dma_start(
                        dbg_avd[:, (2 * m + hh) * TQ:(2 * m + hh + 1) * TQ],
                        sb_avd)
                rd = att.tile([HD, TQ], F32, tag="rd")
                with nc.allow_low_precision(reason="softmax denom"):
                    nc.vector.reciprocal_approx_fast(rd, avd[0:HD, :])
                nc.vector.tensor_tensor(
                    yT[hh * HD:(hh + 1) * HD, m, :], avd[HD:P, :], rd,
                    ALU.mult)

            emit_kq(0)
            for m in range(NPAIR):
                r0 = emit_scores(m, 0)
                r1 = emit_scores(m, 1)
                if m + 1 < NPAIR:
                    emit_kq(m + 1)
                emit_avd(m, 0, r0)
                emit_avd(m, 1, r1)

        if dbg:
            nc.sync.dma_start(dbg_xh[:, :], xh_all.rearrange("p a b -> p (a b)"))
            nc.sync.dma_start(dbg_vdx[:, :], vdx.rearrange("p a b c -> p (a b c)"))
            for m in range(NPAIR):
                nc.sync.dma_start(dbg_k[m * P:(m + 1) * P, :], kTp[m])
                nc.sync.dma_start(dbg_q[m * P:(m + 1) * P, :], qTp[m])
            nc.sync.dma_start(dbg_yT[:, :], yT.rearrange("p a b -> p (a b)"))

        # ================= phase D: Wo + residual =================
        with tc.tile_pool(name="ps_wo", bufs=3, space="PSUM") as ps_wo, \
             tc.tile_pool(name="ps_heatd", bufs=1, space="PSUM") as ps_heatd:
            heat_ps2 = ps_heatd.tile([P, TQ], F32, tag="heatd", name="heatD")

            def heat2(n):
                for _ in range(n):
                    nc.tensor.matmul(heat_ps2, heat_w, heat_src,
                                     start=True, stop=True,
                                     skip_group_check=True)
            for mo in range(CJ):
                pm = ps_wo.tile([P, TQ], F32, tag="wops")
                for k in range(3):
                    nc.tensor.matmul(
                        pm, wo_t[:, 2 * k:2 * k + 2, mo * P:(mo + 1) * P],
                        yT[:, 2 * k:2 * k + 2, :],
                        start=(k == 0), stop=False, perf_mode=DR,
                        skip_group_check=True)
                nc.tensor.matmul(pm, woapp_t[:, mo * P:(mo + 1) * P],
                                 ones_T[:, 0:TQ],
                                 start=False, stop=True, skip_group_check=True)
                nc.vector.scalar_tensor_tensor(
                    x1[mo], pm, 1.0 / SO, own_view(xbf[mo]),
                    ALU.mult, ALU.add)
                heat2(3)

        if dbg:
            for m in range(CJ):
                nc.sync.dma_start(dbg_x1[m * P:(m + 1) * P, :], x1[m])

        # ================= phase E: LN2 =================
        with tc.tile_pool(name="ph_e", bufs=2) as ph_e, \
             tc.tile_pool(name="lne", bufs=1) as lne, \
             tc.tile_pool(name="ps_st2", bufs=1, space="PSUM") as ps_st2, \
             tc.tile_pool(name="ps_bc2", bufs=1, space="PSUM") as ps_bc2, \
             tc.tile_pool(name="ps_heate", bufs=1, space="PSUM") as ps_heate:
            heat_ps3 = ps_heate.tile([P, TQ], F32, tag="heate", name="heatE")

            def heat3(n):
                for _ in range(n):
                    nc.tensor.matmul(heat_ps3, heat_w, heat_src,
                                     start=True, stop=True,
                                     skip_group_check=True)
            mu_ps = ps_st2.tile([1, TQ], F32, tag="mups2", name="mups2")
            sq_ps = ps_st2.tile([1, TQ], F32, tag="sqps2", name="sqps2")
            for m in range(CJ):
                s = ph_e.tile([P, TQ], BF16, tag="x1sq")
                nc.vector.tensor_tensor(s, x1[m], x1[m], ALU.mult)
                nc.tensor.matmul(mu_ps, onesc_bf, x1[m],
                                 start=(m == 0), stop=(m == CJ - 1),
                                 skip_group_check=True)
                nc.tensor.matmul(sq_ps, onesc_bf, s,
                                 start=(m == 0), stop=(m == CJ - 1),
                                 skip_group_check=True)
                heat3(2)
            mu_f = lne.tile([1, TQ], F32, tag="muf2")
            nc.vector.tensor_scalar(mu_f, mu_ps, 1.0 / C, None, ALU.mult)
            var_f = lne.tile([1, TQ], F32, tag="varf2")
            nc.vector.tensor_scalar(var_f, sq_ps, 1.0 / C, None, ALU.mult)
            musq = lne.tile([1, TQ], F32, tag="musq2")
            nc.vector.tensor_tensor(musq, mu_f, mu_f, ALU.mult)
            nc.vector.tensor_tensor(var_f, var_f, musq, ALU.subtract)
            lv_f = lne.tile([1, TQ], F32, tag="lvf2")
            nc.scalar.activation(lv_f, var_f, AF.Ln, bias=eps1)
            rstd_f = lne.tile([1, TQ], F32, tag="rstdf2")
            nc.scalar.activation(rstd_f, lv_f, AF.Exp, scale=-0.5)
            nmr_f = lne.tile([1, TQ], F32, tag="nmrf2")
            nc.vector.tensor_tensor(nmr_f, mu_f, rstd_f, ALU.mult)
            rstd_bf = lne.tile([1, TQ], BF16, tag="rstdbf2")
            nc.vector.tensor_copy(rstd_bf, rstd_f)
            nmr_bf = lne.tile([1, TQ], BF16, tag="nmrbf2")
            nc.vector.tensor_copy(nmr_bf, nmr_f)
            rst_ps = ps_bc2.tile([P, TQ], F32, tag="rstps2", name="rstps2")
            nrm_ps = ps_bc2.tile([P, TQ], F32, tag="nrmps2", name="nrmps2")
            heat3(20)
            nc.tensor.matmul(rst_ps, ones_row, rstd_bf, start=True, stop=True)
            nc.tensor.matmul(nrm_ps, mones_row, nmr_bf, start=True, stop=True)
            heat3(12)
            rst_s = lne.tile([P, TQ], BF16, tag="rsts2")
            nc.vector.tensor_copy(rst_s, rst_ps)
            nrm_s = lne.tile([P, TQ], BF16, tag="nrms2")
            nc.vector.tensor_copy(nrm_s, nrm_ps)
            for m in range(CJ):
                tmp = ph_e.tile([P, TQ], BF16, tag="xnt2")
                nc.vector.tensor_tensor(tmp, x1[m], rst_s, ALU.mult)
                nc.vector.tensor_tensor(xh2[:, m, :], tmp, nrm_s, ALU.add)

        if dbg:
            nc.sync.dma_start(dbg_xh2[:, :], xh2.rearrange("p a b -> p (a b)"))

        # ================= phases F+G: FC + gelu + proj =================
        with tc.tile_pool(name="ph_f", bufs=8) as ph_f, \
             tc.tile_pool(name="ph_g", bufs=2) as ph_g, \
             tc.tile_pool(name="ps_fc", bufs=2, space="PSUM") as ps_fc, \
             tc.tile_pool(name="ps_pj", bufs=1, space="PSUM") as ps_pj:
            pj = ps_pj.tile([P, CJ, TQ], F32, tag="pj", name="pj")
            BLK = 6
            for b0 in range(0, FCJ, BLK):
                xbs, s1s = {}, {}
                for mo in range(b0, b0 + BLK):
                    pf = ps_fc.tile([P, TQ], F32, tag="fc")
                    for k in range(3):
                        nc.tensor.matmul(
                            pf, wfc_t[:, 2 * k:2 * k + 2, mo * P:(mo + 1) * P],
                            xh2[:, 2 * k:2 * k + 2, :],
                            start=(k == 0), stop=(k == 2), perf_mode=DR,
                            skip_group_check=True)
                    xb = ph_f.tile([P, TQ], BF16, tag="xb")
                    nc.vector.tensor_scalar(xb, pf, 1.0 / SF,
                                            bfcb_t[:, mo:mo + 1],
                                            ALU.mult, ALU.add)
                    s1 = ph_f.tile([P, TQ], BF16, tag="s1")
                    nc.scalar.activation(s1, xb, AF.Square)
                    xbs[mo], s1s[mo] = xb, s1
                for mo in range(b0, b0 + BLK):
                    s2 = ph_f.tile([P, TQ], BF16, tag="s2")
                    nc.gpsimd.tensor_tensor(s2, s1s[mo], s1s[mo], ALU.mult)
                    u = ph_f.tile([P, TQ], BF16, tag="u")
                    nc.scalar.activation(u, s2, AF.Sigmoid, scale=2.0 * GA2)
                    nc.vector.tensor_tensor(h1[:, mo, :], xbs[mo], u, ALU.mult)
                    if mo % 2 == 1:
                        r = mo // 2
                        for c in range(CJ):
                            nc.tensor.matmul(
                                pj[:, c, :],
                                wpj_t[:, 2 * r:2 * r + 2, c * P:(c + 1) * P],
                                h1[:, 2 * r:2 * r + 2, :],
                                start=(r == 0), stop=False, perf_mode=DR,
                                skip_group_check=True)
            for c in range(CJ):
                nc.tensor.matmul(pj[:, c, :], pjapp_t[:, c * P:(c + 1) * P],
                                 ones_T[:, 0:TQ],
                                 start=False, stop=True, skip_group_check=True)
                oj = ph_g.tile([P, TQ], F32, tag="oj")
                nc.vector.scalar_tensor_tensor(
                    oj, pj[:, c, :], 1.0 / SP, x1[c],
                    ALU.mult, ALU.add)
                nc.sync.dma_start(out_d[c * P:(c + 1) * P, :], oj)

    nc.compile()
    return nc


def _get_nc():
    if "nc" not in _CACHED:
        _CACHED["nc"] = _build_nc()
    return _CACHED["nc"]


def _perm_blocks(p):
    return [p, 1 - p, 2 + p, 3 - p, 4 + p, 5 - p, 6 + p, 7 - p]


def _fp8(a):
    return np.clip(np.asarray(a, np.float32), -240.0, 240.0).astype(
        ml_dtypes.float8_e4m3)


def _build_in_maps(x, ln1_scale, ln1_bias, Wqkv, bqkv, Wo, bo,
                   ln2_scale, ln2_bias, Wfc, bfc, Wproj, bproj):
    bf16 = ml_dtypes.bfloat16
    x = np.asarray(x, np.float32)
    # Fold LN scale/bias into the following projection (exact):
    Wq64 = np.asarray(ln1_scale, np.float64)[:, None] * np.asarray(Wqkv, np.float64)
    bq64 = np.asarray(bqkv, np.float64) + np.asarray(ln1_bias, np.float64) @ Wq64
    Wfc64 = np.asarray(ln2_scale, np.float64)[:, None] * np.asarray(Wfc, np.float64)
    bfc64 = np.asarray(bfc, np.float64) + np.asarray(ln2_bias, np.float64) @ Wfc64
    colmap = np.arange(3 * C).reshape(H, 3, HD)
    Wq64 = Wq64.astype(np.float32)
    bq64 = bq64.astype(np.float32)

    def pcm(w, scale):  # [C, n] -> fp8 [128, CJ, n], scaled
        n = w.shape[1]
        return np.ascontiguousarray(
            _fp8(np.asarray(w, np.float32).reshape(CJ, P, n)
                 .transpose(1, 0, 2) * scale))

    wq_h = Wq64[:, colmap[:, 0, :].ravel()]
    wk_h = Wq64[:, colmap[:, 1, :].ravel()]
    wv_h = Wq64[:, colmap[:, 2, :].ravel()]
    bq_h = bq64[colmap[:, 0, :].ravel()]
    bk_h = bq64[colmap[:, 1, :].ravel()]
    bv_h = bq64[colmap[:, 2, :].ravel()]

    wq8 = pcm(wq_h, SQ)
    wk8 = pcm(wk_h, SQ)
    wv8 = pcm(wv_h, SV)
    wo8 = pcm(np.asarray(Wo, np.float32), SO)
    wfc8 = pcm(Wfc64.astype(np.float32), SF)
    wpj8 = np.ascontiguousarray(
        _fp8(np.asarray(Wproj, np.float32).reshape(FCJ, P, C)
             .transpose(1, 0, 2) * SP))

    def app2(w8, bias, bscale):
        cs = w8.astype(np.float32).sum((0, 1))      # colsum of scaled fp8 w
        return np.ascontiguousarray(
            np.stack([cs, np.asarray(bias, np.float32) * bscale]).astype(bf16))

    shared = {
        "wv": wv8, "wk": wk8, "wq": wq8, "wo": wo8, "wfc": wfc8, "wpj": wpj8,
        "kapp": app2(wk8, bk_h, SQ),
        "qapp": app2(wq8, bq_h, SQ),
        "vapp": app2(wv8, bv_h, SV),
        "woapp": np.ascontiguousarray(
            (np.asarray(bo, np.float32) * SO)[None, :].astype(bf16)),
        "pjapp": np.ascontiguousarray(
            (np.asarray(bproj, np.float32) * SP)[None, :].astype(bf16)),
        "bfcb": np.ascontiguousarray(
            bfc64.astype(np.float32).reshape(FCJ, P).T),
    }
    in_maps = []
    own_toks = []
    for cidx in range(N_CORES):
        s, p = divmod(cidx, 2)
        blocks = _perm_blocks(p)
        tok = np.concatenate([np.arange(b * P, (b + 1) * P) for b in blocks])
        own = np.concatenate([np.arange(b * P, (b + 1) * P)
                              for b in blocks[0::2]])
        own_toks.append((s, own))
        fp8 = ml_dtypes.float8_e4m3
        tril = (np.arange(P)[None, :] >= np.arange(P)[:, None]).astype(np.float32)
        fl = np.full((P, P), float(p), np.float32)
        on = np.ones((P, P), np.float32)
        msk = np.concatenate([tril, fl, tril, on, fl, tril, fl], axis=1)
        in_maps.append({
            "xt": np.ascontiguousarray(x[s][tok].T.astype(bf16)),
            "msk": np.ascontiguousarray(msk.astype(fp8)),
            **shared,
        })
    return in_maps, own_toks


def kernel(x, ln1_scale, ln1_bias, Wqkv, bqkv, Wo, bo,
           ln2_scale, ln2_bias, Wfc, bfc, Wproj, bproj):
    from concourse.bass_utils import run_bass_kernel_spmd

    in_maps, own_toks = _build_in_maps(
        x, ln1_scale, ln1_bias, Wqkv, bqkv, Wo, bo,
        ln2_scale, ln2_bias, Wfc, bfc, Wproj, bproj)
    nc = _get_nc()
    res = run_bass_kernel_spmd(nc, in_maps, list(range(N_CORES)))

    out = np.empty((B, T, C), np.float32)
    for cidx in range(N_CORES):
        s, own = own_toks[cidx]
        out[s][own] = res.results[cidx]["out"].T
    return out


# revision 32
# speedup vs baseline: 1.0543x; 1.0271x over previous
"""Trainium2 Bass kernel for a GPT-2-style transformer block (v3, fp8).

B=4, T=1024, C=768, H=12 heads (HD=64). 8 NeuronCores.
2 cores per sequence; each core owns 512 query tokens (block-permuted).

v3 vs v2 (bf16 baseline):
- All big matmuls fp8e4 DoubleRow (contraction in 256-pairs): ~1.84x PE.
- Weights host-scaled into fp8 range (x64/x128/x16); scales folded into
  the exp scale, sigmoid scale, and epilogue scalar ops. No unscale ops.
- LN applied as xs = x*rstd only; the -mu*rstd and bias terms enter each
  projection as a K=2 appended matmul (wsum (x) nmr + bias (x) ones).
- Causal masks folded into the score PSUM via appended matmuls
  (tril-bias x identity; per-core flag row x ones) -> no DVE mask work.
- Softmax denominators via fp8 ones-matmuls into a shared [33,TQ] PSUM;
  reciprocal row broadcast to 128 partitions via SBUF->SBUF DMA.
- gelu: xb on DVE, squares on gpsimd, sigmoid on ACT, h1 on DVE.
- proj matmuls interleaved into the FC loop (6-bank persistent PSUM).
"""

import numpy as np
import ml_dtypes

P = 128
B, T, C, H = 4, 1024, 768, 12
HD = C // H        # 64
CJ = C // P        # 6 chunks
NT = T // P        # 8 token blocks
TQ = 512           # own query tokens per core
NQT = TQ // P      # 4 q blocks
FC = 4 * C         # 3072
FCJ = FC // P      # 24
NPAIR = H // 2     # 6 head pairs
GA2 = 0.035677408136300527   # sqrt(2/pi)*0.044715
N_CORES = 8

SQ = 64.0     # q,k weight scale
SV = 16.0     # v weight scale
SO = 128.0    # Wo scale
SF = 64.0     # Wfc scale
SP = 128.0    # Wproj scale
MASKNEG = -1e9

_CACHED = {}


def _build_nc():
    import concourse.bass as bass
    from concourse import bacc, mybir
    import concourse.tile as tile
    from contextlib import ExitStack

    F32 = mybir.dt.float32
    BF16 = mybir.dt.bfloat16
    FP8 = mybir.dt.float8e4
    AF = mybir.ActivationFunctionType
    ALU = mybir.AluOpType
    DR = mybir.MatmulPerfMode.DoubleRow

    nc = bacc.Bacc()

    xt_d = nc.declare_dram_parameter("xt", [C, T], BF16, isOutput=False)
    wv_d = nc.declare_dram_parameter("wv", [P, CJ, C], FP8, isOutput=False)
    wk_d = nc.declare_dram_parameter("wk", [P, CJ, C], FP8, isOutput=False)
    wq_d = nc.declare_dram_parameter("wq", [P, CJ, C], FP8, isOutput=False)
    wo_d = nc.declare_dram_parameter("wo", [P, CJ, C], FP8, isOutput=False)
    wfc_d = nc.declare_dram_parameter("wfc", [P, CJ, FC], FP8, isOutput=False)
    wpj_d = nc.declare_dram_parameter("wpj", [P, FCJ, C], FP8, isOutput=False)
    kapp_d = nc.declare_dram_parameter("kapp", [2, C], BF16, isOutput=False)
    qapp_d = nc.declare_dram_parameter("qapp", [2, C], BF16, isOutput=False)
    vapp_d = nc.declare_dram_parameter("vapp", [2, C], BF16, isOutput=False)
    woapp_d = nc.declare_dram_parameter("woapp", [1, C], BF16, isOutput=False)
    pjapp_d = nc.declare_dram_parameter("pjapp", [1, C], BF16, isOutput=False)
    bfcb_d = nc.declare_dram_parameter("bfcb", [P, FCJ], F32, isOutput=False)
    msk_d = nc.declare_dram_parameter("msk", [P, 2 * P + 384 + 256], FP8, isOutput=False)
    out_d = nc.declare_dram_parameter("out", [C, TQ], F32, isOutput=True)
    import os
    dbg = os.environ.get("KV3_DEBUG") == "1"
    if dbg:
        dbg_xh = nc.declare_dram_parameter("dbg_xh", [P, CJ * T], FP8, isOutput=True)
        dbg_k = nc.declare_dram_parameter("dbg_k", [NPAIR * P, T], BF16, isOutput=True)
        dbg_q = nc.declare_dram_parameter("dbg_q", [NPAIR * P, TQ], BF16, isOutput=True)
        dbg_vdx = nc.declare_dram_parameter("dbg_vdx", [P, NT * H * P], FP8, isOutput=True)
        dbg_yT = nc.declare_dram_parameter("dbg_yT", [P, CJ * TQ], FP8, isOutput=True)
        dbg_x1 = nc.declare_dram_parameter("dbg_x1", [CJ * P, TQ], BF16, isOutput=True)
        dbg_xh2 = nc.declare_dram_parameter("dbg_xh2", [P, CJ * TQ], FP8, isOutput=True)
        dbg_h1 = nc.declare_dram_parameter("dbg_h1", [P, FCJ * TQ], FP8, isOutput=True)
        dbg_avd = nc.declare_dram_parameter("dbg_avd", [P, NPAIR * 2 * TQ], F32, isOutput=True)
        dbg_rd = nc.declare_dram_parameter("dbg_rd", [HD, NPAIR * 2 * TQ], F32, isOutput=True)

    with tile.TileContext(nc) as tc, ExitStack() as ctx:
        persist = ctx.enter_context(tc.tile_pool(name="persist", bufs=1))

        # ---------- DMAs spread across engine queues ----------
        xbf = [persist.tile([P, T], BF16, tag=f"xbf{m}", name=f"xbf{m}")
               for m in range(CJ)]
        for m in range(CJ):
            eng = nc.sync if m % 2 == 0 else nc.scalar
            eng.dma_start(xbf[m], xt_d[m * P:(m + 1) * P, :])
        wv_t = persist.tile([P, CJ, C], FP8, tag="wv")
        nc.sync.dma_start(wv_t, wv_d[:, :, :])
        wk_t = persist.tile([P, CJ, C], FP8, tag="wk")
        nc.sync.dma_start(wk_t, wk_d[:, :, :])
        wq_t = persist.tile([P, CJ, C], FP8, tag="wq")
        nc.scalar.dma_start(wq_t, wq_d[:, :, :])
        wo_t = persist.tile([P, CJ, C], FP8, tag="wo")
        nc.scalar.dma_start(wo_t, wo_d[:, :, :])
        wfc_t = persist.tile([P, CJ, FC], FP8, tag="wfc")
        nc.scalar.dma_start(wfc_t, wfc_d[:, :, :])
        wpj_t = persist.tile([P, FCJ, C], FP8, tag="wpj")
        nc.gpsimd.dma_start(wpj_t, wpj_d[:, :, :])
        kapp_t = persist.tile([2, C], BF16, tag="kapp")
        nc.scalar.dma_start(kapp_t, kapp_d[:, :])
        qapp_t = persist.tile([2, C], BF16, tag="qapp")
        nc.scalar.dma_start(qapp_t, qapp_d[:, :])
        vapp_t = persist.tile([2, C], BF16, tag="vapp")
        nc.scalar.dma_start(vapp_t, vapp_d[:, :])
        woapp_t = persist.tile([1, C], BF16, tag="woapp")
        nc.scalar.dma_start(woapp_t, woapp_d[:, :])
        pjapp_t = persist.tile([1, C], BF16, tag="pjapp")
        nc.scalar.dma_start(pjapp_t, pjapp_d[:, :])
        bfcb_t = persist.tile([P, FCJ], F32, tag="bfcb")
        nc.scalar.dma_start(bfcb_t, bfcb_d[:, :])
        msk_t = persist.tile([P, 2 * P + 384 + 256], FP8, tag="msk")
        nc.scalar.dma_start(msk_t, msk_d[:, :])

        # ---------- constants ----------
        onesc_bf = persist.tile([P, 1], BF16, tag="onescbf")
        nc.vector.memset(onesc_bf, 1.0)
        ones_row = persist.tile([1, P], BF16, tag="onesrow")
        nc.vector.memset(ones_row, 1.0)
        mones_row = persist.tile([1, P], BF16, tag="monesrow")
        nc.vector.memset(mones_row, -1.0)
        ones_hd = persist.tile([1, HD], BF16, tag="oneshd")
        nc.vector.memset(ones_hd, 1.0)
        eps1 = persist.tile([1, 1], F32, tag="eps1")
        nc.vector.memset(eps1, 1e-5)
        warm = persist.tile([1, 1], F32, tag="warm")
        nc.scalar.activation(warm, eps1, AF.Ln)
        # app_rhs rows: 0 = nmr (-mu*rstd, filled by LN1), 1 = ones
        app_rhs = persist.tile([2, T], BF16, tag="apprhs")
        nc.vector.memset(app_rhs, 1.0)
        ones_T = persist.tile([1, T], BF16, tag="onesT")
        nc.vector.memset(ones_T, 1.0)
        heat_w = persist.tile([P, P], BF16, tag="heatw")
        nc.vector.memset(heat_w, 0.001)
        heat_src = persist.tile([P, TQ], BF16, tag="heatsrc")
        nc.vector.memset(heat_src, 0.001)

        xh_all = persist.tile([P, CJ, T], FP8, tag="xhall")
        xso = persist.tile([P, CJ, TQ], FP8, tag="xso")
        vdx = persist.tile([P, NT, H, P], FP8, tag="vdx")
        nc.vector.memset(vdx.rearrange("p t h (a d) -> p t h a d", a=2)[:, :, :, 0, :], SV)
        kTp = [persist.tile([P, T], BF16, tag=f"kTp{m}", name=f"kTp{m}")
               for m in range(NPAIR)]
        qTp = [persist.tile([P, TQ], BF16, tag=f"qTp{m}", name=f"qTp{m}")
               for m in range(NPAIR)]
        yT = persist.tile([P, CJ, TQ], FP8, tag="yT")
        x1 = [persist.tile([P, TQ], BF16, tag=f"x1{m}", name=f"x1{m}")
              for m in range(CJ)]
        xh2 = persist.tile([P, CJ, TQ], FP8, tag="xh2")
        h1 = persist.tile([P, FCJ, TQ], FP8, tag="h1")

        def own_view(t):
            """[P, NQT, P] even-block (own query) view of a [P, T] tile."""
            return t.rearrange("p (b c) -> p b c", c=P)[:, 0::2, :]

        # ================= phase A: LN1 =================
        with tc.tile_pool(name="ph_a", bufs=2) as ph_a, \
             tc.tile_pool(name="lna", bufs=1) as lna, \
             tc.tile_pool(name="ps_st", bufs=1, space="PSUM") as ps_st, \
             tc.tile_pool(name="ps_bc", bufs=1, space="PSUM") as ps_bc, \
             tc.tile_pool(name="ps_heat", bufs=1, space="PSUM") as ps_heat:
            heat_ps = ps_heat.tile([P, TQ], F32, tag="heat", name="heatA")

            def heat(n):
                for _ in range(n):
                    nc.tensor.matmul(heat_ps, heat_w, heat_src,
                                     start=True, stop=True,
                                     skip_group_check=True)
            heat(20)
            mu_ps = ps_st.tile([1, 2, TQ], F32, tag="mups", name="mups")
            sq_ps = ps_st.tile([1, 2, TQ], F32, tag="sqps", name="sqps")
            for m in range(CJ):
                b = xbf[m]
                s = ph_a.tile([P, T], BF16, tag="xsq")
                nc.vector.tensor_tensor(s, b, b, ALU.mult)
                for hf in range(2):
                    nc.tensor.matmul(mu_ps[:, hf, :], onesc_bf,
                                     b[:, hf * TQ:(hf + 1) * TQ],
                                     start=(m == 0), stop=(m == CJ - 1),
                                     skip_group_check=True)
                for hf in range(2):
                    nc.tensor.matmul(sq_ps[:, hf, :], onesc_bf,
                                     s[:, hf * TQ:(hf + 1) * TQ],
                                     start=(m == 0), stop=(m == CJ - 1),
                                     skip_group_check=True)
                heat(2)
            mu_f = lna.tile([1, T], F32, tag="muf")
            nc.vector.tensor_scalar(mu_f, mu_ps.rearrange("o a b -> o (a b)"),
                                    1.0 / C, None, ALU.mult)
            var_f = lna.tile([1, T], F32, tag="varf")
            nc.vector.tensor_scalar(var_f, sq_ps.rearrange("o a b -> o (a b)"),
                                    1.0 / C, None, ALU.mult)
            musq = lna.tile([1, T], F32, tag="musq")
            nc.vector.tensor_tensor(musq, mu_f, mu_f, ALU.mult)
            nc.vector.tensor_tensor(var_f, var_f, musq, ALU.subtract)
            # rstd = exp(-0.5*ln(var+eps)) keeps ACT on the ln/exp table set
            lv_f = lna.tile([1, T], F32, tag="lvf")
            nc.scalar.activation(lv_f, var_f, AF.Ln, bias=eps1)
            rstd_f = lna.tile([1, T], F32, tag="rstdf")
            nc.scalar.activation(rstd_f, lv_f, AF.Exp, scale=-0.5)
            nmr_f = lna.tile([1, T], F32, tag="nmrf")
            nc.vector.tensor_tensor(nmr_f, mu_f, rstd_f, ALU.mult)
            # app_rhs row 0 = -mu*rstd
            nc.vector.tensor_scalar(app_rhs[0:1, :], nmr_f, -1.0, None, ALU.mult)
            rstd_bf = lna.tile([1, T], BF16, tag="rstdbf")
            nc.vector.tensor_copy(rstd_bf, rstd_f)
            # broadcast rstd across partitions via K=1 matmuls
            rst_ps = ps_bc.tile([P, 2, TQ], F32, tag="rstps", name="rstps")
            for hf in range(2):
                nc.tensor.matmul(rst_ps[:, hf, :], ones_row,
                                 rstd_bf[:, hf * TQ:(hf + 1) * TQ],
                                 start=True, stop=True)
            heat(20)
            rst_s = lna.tile([P, T], BF16, tag="rsts")
            nc.vector.tensor_copy(rst_s, rst_ps.rearrange("p a b -> p (a b)"))
            # xs = x * rstd  (the -mu*rstd part is folded into each proj)
            for m in range(CJ):
                nc.vector.tensor_tensor(xh_all[:, m, :], xbf[m], rst_s,
                                        ALU.mult)
                heat(2)
        # own-query columns of xs, contiguous for the Q matmuls
        nc.gpsimd.dma_start(
            xso.rearrange("p c (b q) -> p c b q", q=P),
            xh_all.rearrange("p c (b q) -> p c b q", q=P)[:, :, 0::2, :])

        # ================= phase B: V =================
        with tc.tile_pool(name="ps_v", bufs=2, space="PSUM") as ps_v, \
             tc.tile_pool(name="ps_heatb", bufs=1, space="PSUM") as ps_heatb:
            heat_psb = ps_heatb.tile([P, TQ], F32, tag="heatb", name="heatB")
            for t in range(NT):
                for _ in range(2):
                    nc.tensor.matmul(heat_psb, heat_w, heat_src,
                                     start=True, stop=True,
                                     skip_group_check=True)
                pv = ps_v.tile([P, 2, TQ], F32, tag="pv")
                for k in range(3):
                    lhs = xh_all[:, 2 * k:2 * k + 2, t * P:(t + 1) * P]
                    for half in range(2):
                        nc.tensor.matmul(
                            pv[:, half, 0:384], lhs,
                            wv_t[:, 2 * k:2 * k + 2,
                                 half * 384:(half + 1) * 384],
                            start=(k == 0), stop=False, perf_mode=DR,
                            skip_group_check=True)
                for half in range(2):
                    nc.tensor.matmul(
                        pv[:, half, 0:384],
                        app_rhs[:, t * P:(t + 1) * P],
                        vapp_t[:, half * 384:(half + 1) * 384],
                        start=False, stop=True, skip_group_check=True)
                nc.vector.tensor_copy(
                    vdx.rearrange("p t (g s) (a d) -> p t g s a d",
                                  g=2, a=2)[:, t, :, :, 1, :],
                    pv[:, :, 0:384].rearrange("p a (h d) -> p a h d", d=HD))

        # ================= phase C: K, Q, attention (sw-pipelined) =========
        with tc.tile_pool(name="att", bufs=6) as att, \
             tc.tile_pool(name="ps_kq", bufs=2, space="PSUM") as ps_kq, \
             tc.tile_pool(name="ps_sc", bufs=2, space="PSUM") as ps_sc, \
             tc.tile_pool(name="ps_avd", bufs=2, space="PSUM") as ps_avd:

            def emit_kq(m):
                pks = [ps_kq.tile([P, TQ], F32, tag="kq", name=f"pk{m}_{tk}")
                       for tk in range(2)]
                for k in range(3):
                    for tk in range(2):
                        nc.tensor.matmul(
                            pks[tk], wk_t[:, 2 * k:2 * k + 2, m * P:(m + 1) * P],
                            xh_all[:, 2 * k:2 * k + 2, tk * TQ:(tk + 1) * TQ],
                            start=(k == 0), stop=False, perf_mode=DR,
                            skip_group_check=True)
                for tk in range(2):
                    nc.tensor.matmul(
                        pks[tk], kapp_t[:, m * P:(m + 1) * P],
                        app_rhs[:, tk * TQ:(tk + 1) * TQ],
                        start=False, stop=True, skip_group_check=True)
                    nc.vector.tensor_copy(kTp[m][:, tk * TQ:(tk + 1) * TQ],
                                          pks[tk])
                pq = ps_kq.tile([P, TQ], F32, tag="kq", name=f"pq{m}")
                for k in range(3):
                    nc.tensor.matmul(
                        pq, wq_t[:, 2 * k:2 * k + 2, m * P:(m + 1) * P],
                        xso[:, 2 * k:2 * k + 2, :],
                        start=(k == 0), stop=False, perf_mode=DR,
                        skip_group_check=True)
                nc.tensor.matmul(
                    pq, qapp_t[:, m * P:(m + 1) * P],
                    app_rhs.rearrange("r (b q) -> r b q", q=P)[:, 0::2, :],
                    start=False, stop=True, skip_group_check=True)
                nc.vector.tensor_copy(qTp[m], pq)

            def emit_scores(m, hh):
                hs = slice(hh * HD, (hh + 1) * HD)
                exs = []
                for j in range(2):
                    w = TQ - j * P
                    sc = ps_sc.tile([P, 2, TQ], F32, tag="sc")
                    for i in range(2):
                        nc.tensor.matmul(
                            sc[:, i, 0:w],
                            kTp[m][hs, (2 * j + i) * P:(2 * j + i + 1) * P],
                            qTp[m][hs, j * P:TQ],
                            start=True, stop=True, skip_group_check=True)
                    ex = att.tile([P, 2, TQ], FP8, tag="ex")
                    nc.scalar.activation(ex[:, :, 0:w], sc[:, :, 0:w],
                                         AF.Exp, scale=0.125 / (SQ * SQ))
                    eng = nc.vector if j == 0 else nc.gpsimd
                    eng.tensor_tensor(
                        ex[:, :, 0:P], ex[:, :, 0:P],
                        msk_t[:, 0:2 * P].rearrange("p (a b) -> p a b", a=2),
                        ALU.mult)
                    exs.append(ex)
                sc = ps_sc.tile([P, 2, TQ], F32, tag="sc")
                for i in range(2):
                    nc.tensor.matmul(
                        sc[:, 0, i * 256:i * 256 + 256],
                        kTp[m][hs, (4 + i) * P:(5 + i) * P],
                        qTp[m][hs, 2 * P:TQ],
                        start=True, stop=True, skip_group_check=True)
                    nc.tensor.matmul(
                        sc[:, 1, i * P:(i + 1) * P],
                        kTp[m][hs, (6 + i) * P:(7 + i) * P],
                        qTp[m][hs, 3 * P:TQ],
                        start=True, stop=True, skip_group_check=True)
                for _ in range(2):
                    nc.tensor.matmul(sc[:, 1, 2 * P:TQ], heat_w,
                                     heat_src[:, 0:2 * P],
                                     start=True, stop=True,
                                     skip_group_check=True)
                ex23 = att.tile([P, 2, TQ], FP8, tag="ex")
                nc.scalar.activation(ex23[:, 0, :], sc[:, 0, :],
                                     AF.Exp, scale=0.125 / (SQ * SQ))
                nc.scalar.activation(ex23[:, 1, 0:256], sc[:, 1, 0:256],
                                     AF.Exp, scale=0.125 / (SQ * SQ))
                nc.gpsimd.tensor_tensor(
                    ex23[:, 0, 0:384], ex23[:, 0, 0:384],
                    msk_t[:, 2 * P:2 * P + 384], ALU.mult)
                nc.vector.tensor_tensor(
                    ex23[:, 1, 0:256], ex23[:, 1, 0:256],
                    msk_t[:, 2 * P + 384:2 * P + 640], ALU.mult)
                exs.append(ex23)
                return exs

            def emit_avd(m, hh, exs):
                h = 2 * m + hh
                avd = ps_avd.tile([P, TQ], F32, tag="avd", name=f"avd{m}_{hh}")

                def av_rhs(j):
                    if j < 2:
                        return exs[j][:, :, 0:TQ - j * P]
                    if j == 2:
                        return exs[2].rearrange(
                            "p a (b c) -> p (a b) c", c=256)[:, 0:2, :]
                    return exs[2].rearrange(
                        "p a (b c) -> p (a b) c", c=P)[:, 4:6, :]
                for j in range(NQT):
                    nc.tensor.matmul(
                        avd[:, j * P:TQ], vdx[:, 2 * j:2 * j + 2, h, :],
                        av_rhs(j), start=(j == 0), stop=(j == NQT - 1),
                        perf_mode=DR, skip_group_check=True)
                if dbg:
                    sb_avd = att.tile([P, TQ], F32, tag="sbavd")
                    nc.vector.tensor_copy(sb_avd, avd)
                    nc.sync.dma_start(
                        dbg_avd[:, (2 * m + hh) * TQ:(2 * m + hh + 1) * TQ],
                        sb_avd)
                rd = att.tile([HD, TQ], F32, tag="rd")
                with nc.allow_low_precision(reason="softmax denom"):
                    nc.vector.reciprocal_approx_fast(rd, avd[0:HD, :])
                nc.vector.tensor_tensor(
                    yT[hh * HD:(hh + 1) * HD, m, :], avd[HD:P, :], rd,
                    ALU.mult)

            emit_kq(0)
            for m in range(NPAIR):
                r0 = emit_scores(m, 0)
                r1 = emit_scores(m, 1)
                if m + 1 < NPAIR:
                    emit_kq(m + 1)
                emit_avd(m, 0, r0)
                emit_avd(m, 1, r1)

        if dbg:
            nc.sync.dma_start(dbg_xh[:, :], xh_all.rearrange("p a b -> p (a b)"))
            nc.sync.dma_start(dbg_vdx[:, :], vdx.rearrange("p a b c -> p (a b c)"))
            for m in range(NPAIR):
                nc.sync.dma_start(dbg_k[m * P:(m + 1) * P, :], kTp[m])
                nc.sync.dma_start(dbg_q[m * P:(m + 1) * P, :], qTp[m])
            nc.sync.dma_start(dbg_yT[:, :], yT.rearrange("p a b -> p (a b)"))

        # ================= phase D: Wo + residual =================
        with tc.tile_pool(name="ps_wo", bufs=3, space="PSUM") as ps_wo, \
             tc.tile_pool(name="ps_heatd", bufs=1, space="PSUM") as ps_heatd:
            heat_ps2 = ps_heatd.tile([P, TQ], F32, tag="heatd", name="heatD")

            def heat2(n):
                for _ in range(n):
                    nc.tensor.matmul(heat_ps2, heat_w, heat_src,
                                     start=True, stop=True,
                                     skip_group_check=True)
            for mo in range(CJ):
                pm = ps_wo.tile([P, TQ], F32, tag="wops")
                for k in range(3):
                    nc.tensor.matmul(
                        pm, wo_t[:, 2 * k:2 * k + 2, mo * P:(mo + 1) * P],
                        yT[:, 2 * k:2 * k + 2, :],
                        start=(k == 0), stop=False, perf_mode=DR,
                        skip_group_check=True)
                nc.tensor.matmul(pm, woapp_t[:, mo * P:(mo + 1) * P],
                                 ones_T[:, 0:TQ],
                                 start=False, stop=True, skip_group_check=True)
                nc.vector.scalar_tensor_tensor(
                    x1[mo], pm, 1.0 / SO, own_view(xbf[mo]),
                    ALU.mult, ALU.add)
                heat2(3)

        if dbg:
            for m in range(CJ):
                nc.sync.dma_start(dbg_x1[m * P:(m + 1) * P, :], x1[m])

        # ================= phase E: LN2 =================
        with tc.tile_pool(name="ph_e", bufs=2) as ph_e, \
             tc.tile_pool(name="lne", bufs=1) as lne, \
             tc.tile_pool(name="ps_st2", bufs=1, space="PSUM") as ps_st2, \
             tc.tile_pool(name="ps_bc2", bufs=1, space="PSUM") as ps_bc2, \
             tc.tile_pool(name="ps_heate", bufs=1, space="PSUM") as ps_heate:
            heat_ps3 = ps_heate.tile([P, TQ], F32, tag="heate", name="heatE")

            def heat3(n):
                for _ in range(n):
                    nc.tensor.matmul(heat_ps3, heat_w, heat_src,
                                     start=True, stop=True,
                                     skip_group_check=True)
            mu_ps = ps_st2.tile([1, TQ], F32, tag="mups2", name="mups2")
            sq_ps = ps_st2.tile([1, TQ], F32, tag="sqps2", name="sqps2")
            for m in range(CJ):
                s = ph_e.tile([P, TQ], BF16, tag="x1sq")
                nc.vector.tensor_tensor(s, x1[m], x1[m], ALU.mult)
                nc.tensor.matmul(mu_ps, onesc_bf, x1[m],
                                 start=(m == 0), stop=(m == CJ - 1),
                                 skip_group_check=True)
                nc.tensor.matmul(sq_ps, onesc_bf, s,
                                 start=(m == 0), stop=(m == CJ - 1),
                                 skip_group_check=True)
                heat3(2)
            mu_f = lne.tile([1, TQ], F32, tag="muf2")
            nc.vector.tensor_scalar(mu_f, mu_ps, 1.0 / C, None, ALU.mult)
            var_f = lne.tile([1, TQ], F32, tag="varf2")
            nc.vector.tensor_scalar(var_f, sq_ps, 1.0 / C, None, ALU.mult)
            musq = lne.tile([1, TQ], F32, tag="musq2")
            nc.vector.tensor_tensor(musq, mu_f, mu_f, ALU.mult)
            nc.vector.tensor_tensor(var_f, var_f, musq, ALU.subtract)
            lv_f = lne.tile([1, TQ], F32, tag="lvf2")
            nc.scalar.activation(lv_f, var_f, AF.Ln, bias=eps1)
            rstd_f = lne.tile([1, TQ], F32, tag="rstdf2")
            nc.scalar.activation(rstd_f, lv_f, AF.Exp, scale=-0.5)
            nmr_f = lne.tile([1, TQ], F32, tag="nmrf2")
            nc.vector.tensor_tensor(nmr_f, mu_f, rstd_f, ALU.mult)
            rstd_bf = lne.tile([1, TQ], BF16, tag="rstdbf2")
            nc.vector.tensor_copy(rstd_bf, rstd_f)
            nmr_bf = lne.tile([1, TQ], BF16, tag="nmrbf2")
            nc.vector.tensor_copy(nmr_bf, nmr_f)
            rst_ps = ps_bc2.tile([P, TQ], F32, tag="rstps2", name="rstps2")
            nrm_ps = ps_bc2.tile([P, TQ], F32, tag="nrmps2", name="nrmps2")
            heat3(20)
            nc.tensor.matmul(rst_ps, ones_row, rstd_bf, start=True, stop=True)
            nc.tensor.matmul(nrm_ps, mones_row, nmr_bf, start=True, stop=True)
            heat3(12)
            rst_s = lne.tile([P, TQ], BF16, tag="rsts2")
            nc.vector.tensor_copy(rst_s, rst_ps)
            nrm_s = lne.tile([P, TQ], BF16, tag="nrms2")
            nc.vector.tensor_copy(nrm_s, nrm_ps)
            for m in range(CJ):
                tmp = ph_e.tile([P, TQ], BF16, tag="xnt2")
                nc.vector.tensor_tensor(tmp, x1[m], rst_s, ALU.mult)
                nc.vector.tensor_tensor(xh2[:, m, :], tmp, nrm_s, ALU.add)

        if dbg:
            nc.sync.dma_start(dbg_xh2[:, :], xh2.rearrange("p a b -> p (a b)"))

        # ================= phases F+G: FC + gelu + proj =================
        with tc.tile_pool(name="ph_f", bufs=8) as ph_f, \
             tc.tile_pool(name="ph_g", bufs=2) as ph_g, \
             tc.tile_pool(name="ps_fc", bufs=2, space="PSUM") as ps_fc, \
             tc.tile_pool(name="ps_pj", bufs=1, space="PSUM") as ps_pj:
            pj = ps_pj.tile([P, CJ, TQ], F32, tag="pj", name="pj")
            BLK = 6
            for b0 in range(0, FCJ, BLK):
                xbs, s1s = {}, {}
                for mo in range(b0, b0 + BLK):
                    pf = ps_fc.tile([P, TQ], F32, tag="fc")
                    for k in range(3):
                        nc.tensor.matmul(
                            pf, wfc_t[:, 2 * k:2 * k + 2, mo * P:(mo + 1) * P],
                            xh2[:, 2 * k:2 * k + 2, :],
                            start=(k == 0), stop=(k == 2), perf_mode=DR,
                            skip_group_check=True)
                    xb = ph_f.tile([P, TQ], BF16, tag="xb")
                    nc.vector.tensor_scalar(xb, pf, 1.0 / SF,
                                            bfcb_t[:, mo:mo + 1],
                                            ALU.mult, ALU.add)
                    s1 = ph_f.tile([P, TQ], BF16, tag="s1")
                    nc.scalar.activation(s1, xb, AF.Square)
                    xbs[mo], s1s[mo] = xb, s1
                for mo in range(b0, b0 + BLK):
                    s2 = ph_f.tile([P, TQ], BF16, tag="s2")
                    nc.gpsimd.tensor_tensor(s2, s1s[mo], s1s[mo], ALU.mult)
                    u = ph_f.tile([P, TQ], BF16, tag="u")
                    nc.scalar.activation(u, s2, AF.Sigmoid, scale=2.0 * GA2)
                    nc.vector.tensor_tensor(h1[:, mo, :], xbs[mo], u, ALU.mult)
                    if mo % 2 == 1:
                        r = mo // 2
                        for c in range(CJ):
                            nc.tensor.matmul(
                                pj[:, c, :],
                                wpj_t[:, 2 * r:2 * r + 2, c * P:(c + 1) * P],
                                h1[:, 2 * r:2 * r + 2, :],
                                start=(r == 0), stop=False, perf_mode=DR,
                                skip_group_check=True)
            for c in range(CJ):
                nc.tensor.matmul(pj[:, c, :], pjapp_t[:, c * P:(c + 1) * P],
                                 ones_T[:, 0:TQ],
                                 start=False, stop=True, skip_group_check=True)
                oj = ph_g.tile([P, TQ], F32, tag="oj")
                nc.vector.scalar_tensor_tensor(
                    oj, pj[:, c, :], 1.0 / SP, x1[c],
                    ALU.mult, ALU.add)
                nc.sync.dma_start(out_d[c * P:(c + 1) * P, :], oj)

    nc.compile()
    return nc


def _get_nc():
    if "nc" not in _CACHED:
        _CACHED["nc"] = _build_nc()
    return _CACHED["nc"]


def _perm_blocks(p):
    return [p, 1 - p, 2 + p, 3 - p, 4 + p, 5 - p, 6 + p, 7 - p]


def _fp8(a):
    return np.clip(np.asarray(a, np.float32), -240.0, 240.0).astype(
        ml_dtypes.float8_e4m3)


def _build_in_maps(x, ln1_scale, ln1_bias, Wqkv, bqkv, Wo, bo,
                   ln2_scale, ln2_bias, Wfc, bfc, Wproj, bproj):
    bf16 = ml_dtypes.bfloat16
    x = np.asarray(x, np.float32)
    # Fold LN scale/bias into the following projection (exact):
    Wq64 = np.asarray(ln1_scale, np.float64)[:, None] * np.asarray(Wqkv, np.float64)
    bq64 = np.asarray(bqkv, np.float64) + np.asarray(ln1_bias, np.float64) @ Wq64
    Wfc64 = np.asarray(ln2_scale, np.float64)[:, None] * np.asarray(Wfc, np.float64)
    bfc64 = np.asarray(bfc, np.float64) + np.asarray(ln2_bias, np.float64) @ Wfc64
    colmap = np.arange(3 * C).reshape(H, 3, HD)
    Wq64 = Wq64.astype(np.float32)
    bq64 = bq64.astype(np.float32)

    def pcm(w, scale):  # [C, n] -> fp8 [128, CJ, n], scaled
        n = w.shape[1]
        return np.ascontiguousarray(
            _fp8(np.asarray(w, np.float32).reshape(CJ, P, n)
                 .transpose(1, 0, 2) * scale))

    wq_h = Wq64[:, colmap[:, 0, :].ravel()]
    wk_h = Wq64[:, colmap[:, 1, :].ravel()]
    wv_h = Wq64[:, colmap[:, 2, :].ravel()]
    bq_h = bq64[colmap[:, 0, :].ravel()]
    bk_h = bq64[colmap[:, 1, :].ravel()]
    bv_h = bq64[colmap[:, 2, :].ravel()]

    wq8 = pcm(wq_h, SQ)
    wk8 = pcm(wk_h, SQ)
    wv8 = pcm(wv_h, SV)
    wo8 = pcm(np.asarray(Wo, np.float32), SO)
    wfc8 = pcm(Wfc64.astype(np.float32), SF)
    wpj8 = np.ascontiguousarray(
        _fp8(np.asarray(Wproj, np.float32).reshape(FCJ, P, C)
             .transpose(1, 0, 2) * SP))

    def app2(w8, bias, bscale):
        cs = w8.astype(np.float32).sum((0, 1))      # colsum of scaled fp8 w
        return np.ascontiguousarray(
            np.stack([cs, np.asarray(bias, np.float32) * bscale]).astype(bf16))

    shared = {
        "wv": wv8, "wk": wk8, "wq": wq8, "wo": wo8, "wfc": wfc8, "wpj": wpj8,
        "kapp": app2(wk8, bk_h, SQ),
        "qapp": app2(wq8, bq_h, SQ),
        "vapp": app2(wv8, bv_h, SV),
        "woapp": np.ascontiguousarray(
            (np.asarray(bo, np.float32) * SO)[None, :].astype(bf16)),
        "pjapp": np.ascontiguousarray(
            (np.asarray(bproj, np.float32) * SP)[None, :].astype(bf16)),
        "bfcb": np.ascontiguousarray(
            bfc64.astype(np.float32).reshape(FCJ, P).T),
    }
    in_maps = []
    own_toks = []
    for cidx in range(N_CORES):
        s, p = divmod(cidx, 2)
        blocks = _perm_blocks(p)
        tok = np.concatenate([np.arange(b * P, (b + 1) * P) for b in blocks])
        own = np.concatenate([np.arange(b * P, (b + 1) * P)
                              for b in blocks[0::2]])
        own_toks.append((s, own))
        fp8 = ml_dtypes.float8_e4m3
        tril = (np.arange(P)[None, :] >= np.arange(P)[:, None]).astype(np.float32)
        fl = np.full((P, P), float(p), np.float32)
        on = np.ones((P, P), np.float32)
        msk = np.concatenate([tril, fl, tril, on, fl, tril, fl], axis=1)
        in_maps.append({
            "xt": np.ascontiguousarray(x[s][tok].T.astype(bf16)),
            "msk": np.ascontiguousarray(msk.astype(fp8)),
            **shared,
        })
    return in_maps, own_toks


def kernel(x, ln1_scale, ln1_bias, Wqkv, bqkv, Wo, bo,
           ln2_scale, ln2_bias, Wfc, bfc, Wproj, bproj):
    from concourse.bass_utils import run_bass_kernel_spmd

    in_maps, own_toks = _build_in_maps(
        x, ln1_scale, ln1_bias, Wqkv, bqkv, Wo, bo,
        ln2_scale, ln2_bias, Wfc, bfc, Wproj, bproj)
    nc = _get_nc()
    res = run_bass_kernel_spmd(nc, in_maps, list(range(N_CORES)))

    out = np.empty((B, T, C), np.float32)
    for cidx in range(N_CORES):
        s, own = own_toks[cidx]
        out[s][own] = res.results[cidx]["out"].T
    return out
